# revision 11
# baseline (speedup 1.0000x reference)
"""HGT link predictor on 8 Trainium2 NeuronCores (Bass/Tile SPMD kernel).

Strategy (hardcoded for nn_HGTLinkPredictor, N=50000 E=800000 P=100000 C=128 H=4 D=32):
 - Shard dst nodes (and their incoming edges) across 8 cores in contiguous
   128-node blocks. Edges sorted by dst on host.
 - Per layer: each core computes q/k/v projections for its node shard from a
   host-transposed feature tile (relation transforms, attention scale and the
   sigmoid-skip coefficients are all folded into the weights on host), writes
   packed [k|v] fp16 rows, AllGathers them so every core can fetch k/v of any
   src node with a single 512B-descriptor indirect DMA per edge (one packed
   row instead of separate k/v/q fetches).
 - q[dst] rows never leave SBUF: per 128-edge tile a one-hot matmul
   (S2^T @ Q_block) on the tensor engine expands the block's q rows to edge
   slots, with the transposed one-hot streamed from a host-precomputed table.
 - Attention logits via fp16 elementwise mult + segmented reduce; exp on the
   scalar engine into a fused [ex | v*ex] bf16 tile so a single 132-column
   matmul per 128-edge tile accumulates both the softmax denominator and the
   weighted value sum in PSUM.
 - Gelu + output projection run as a deferred second pass (one activation
   table load), producing transposed features directly so layer-2 projections
   need no PE transposes. Link decode is one tiny matmul per block; final
   per-edge logits are assembled host-side from per-node partial sums.
"""

import math
import numpy as np
import ml_dtypes
from contextlib import ExitStack

import concourse.bass as bass
import concourse.tile as tile
from concourse import bacc, mybir
from concourse import bass_utils
from concourse.masks import make_identity

F32 = mybir.dt.float32
F16 = mybir.dt.float16
BF16 = mybir.dt.bfloat16
I32 = mybir.dt.int32
AF = mybir.ActivationFunctionType
OP = mybir.AluOpType

CORES = 8
EPS = 1e-30
def _expand_last(ap, n):
    """Append a step-0 (broadcast) innermost dim of size n to an AP."""
    new = [list(p) for p in ap.ap] + [[0, n]]
    return bass.AP(ap.tensor, ap.offset, new)


def _apn(ap, dims, off=0):
    """AP keeping the partition dim of `ap` but custom free-dim pattern."""
    return bass.AP(ap.tensor, ap.offset + off,
                   [list(ap.ap[0])] + [list(d) for d in dims])


# ----------------------------------------------------------------- host prep

def _host_prep(x, edge_index):
    N, C = x.shape
    E = edge_index.shape[1]

    NPC = int(math.ceil(N / (CORES * 128))) * 128   # nodes per core (padded)
    BPC = NPC // 128                                # blocks per core
    NPAD = NPC * CORES

    src = edge_index[0].astype(np.int64)
    dst = edge_index[1].astype(np.int64)
    order = np.argsort(dst, kind="stable")
    s_src, s_dst = src[order], dst[order]

    core_of = s_dst // NPC
    blk_of = (s_dst % NPC) // 128
    gblk = core_of * BPC + blk_of

    cnt = np.zeros((CORES, BPC), dtype=np.int64)
    np.add.at(cnt, (core_of, blk_of), 1)
    T_b = np.maximum(1, np.ceil(cnt.max(axis=0) / 128).astype(np.int64))
    tiles_total = int(T_b.sum())

    blk_starts = np.concatenate([[0], np.cumsum(T_b)])[:-1] * 128
    grp_start = np.zeros(CORES * BPC + 1, dtype=np.int64)
    np.add.at(grp_start, gblk + 1, 1)
    grp_start = np.cumsum(grp_start)
    pos_in_grp = np.arange(E) - grp_start[gblk]

    cap = tiles_total * 128
    ekv = np.zeros((CORES, cap), dtype=np.int32)     # global src node id
    eslot = np.full((CORES, cap), -1.0, dtype=np.float32)

    flat_pos = blk_starts[blk_of] + pos_in_grp
    ekv[core_of, flat_pos] = s_src.astype(np.int32)
    eslot[core_of, flat_pos] = (s_dst % 128).astype(np.float32)

    # transposed one-hot: S2[j, t*128+e] = (eslot[t*128+e] == j), bf16
    s2 = (eslot.reshape(CORES, 1, cap)
          == np.arange(128, dtype=np.float32)[None, :, None])
    s2 = s2.astype(ml_dtypes.bfloat16)
    # [128, tiles_total] partition-major: entry [p, t] = edge t*128+p
    ekv = ekv.reshape(CORES, tiles_total, 128).transpose(0, 2, 1).copy()
    eslot = np.ascontiguousarray(
        eslot.reshape(CORES, tiles_total, 128).transpose(0, 2, 1)
    ).astype(ml_dtypes.bfloat16)

    meta = dict(N=N, C=C, E=E, NPC=NPC, BPC=BPC, NPAD=NPAD,
                T_b=tuple(int(t) for t in T_b), tiles_total=tiles_total)
    arrays = dict(ekv=ekv, eslot=eslot, s2=s2)
    return meta, arrays


def _prep_weights(inputs):
    """Fold relation transforms, attention scale and skip gates into weights.

    Stored features are pre-scaled: x_stored = (1-a1)*x, h1_stored = (1-a2)*h1,
    so the skip connection becomes a plain add and the projection weights are
    divided by the input scale.
    """
    C = inputs["W1k"].shape[0]
    H, D = inputs["a1"].shape[0], inputs["a1"].shape[1]
    a_s = {l: float(1.0 / (1.0 + np.exp(-float(np.asarray(inputs[f"skip{l}"])))))
           for l in (1, 2)}
    out = {"asig1": a_s[1], "asig2": a_s[2]}
    names = []
    for l in (1, 2):
        a_rel = np.asarray(inputs[f"a{l}"], np.float64)
        m_rel = np.asarray(inputs[f"m{l}"], np.float64)
        p_rel = np.asarray(inputs[f"p{l}"], np.float64)
        A = np.zeros((C, C)); M = np.zeros((C, C))
        for h in range(H):
            A[h * D:(h + 1) * D, h * D:(h + 1) * D] = a_rel[h]
            M[h * D:(h + 1) * D, h * D:(h + 1) * D] = m_rel[h]
        qscale = np.repeat(p_rel / np.sqrt(D), D)
        in_scale = 1.0 - a_s[l]
        Wq = np.asarray(inputs[f"W{l}q"], np.float64) * qscale / in_scale
        Wk = np.asarray(inputs[f"W{l}k"], np.float64) @ A / in_scale
        Wv = np.asarray(inputs[f"W{l}v"], np.float64) @ M / in_scale
        bq = np.asarray(inputs[f"b{l}q"], np.float64) * qscale
        bk = np.asarray(inputs[f"b{l}k"], np.float64) @ A
        bv = np.asarray(inputs[f"b{l}v"], np.float64) @ M
        out_scale = a_s[1] * (1.0 - a_s[2]) if l == 1 else a_s[2]
        Wo = np.asarray(inputs[f"Wo{l}"], np.float64) * out_scale
        boa = np.asarray(inputs[f"bo{l}"], np.float64) * out_scale
        out[f"Wq{l}"] = Wq.astype(np.float16)
        out[f"Wk{l}"] = Wk.astype(np.float16)
        out[f"Wv{l}"] = Wv.astype(np.float16)
        out[f"Wo{l}"] = Wo.astype(np.float16)
        out[f"bq{l}"] = np.broadcast_to(bq.astype(np.float32), (128, C)).copy()
        out[f"bk{l}"] = np.broadcast_to(bk.astype(np.float32), (128, C)).copy()
        out[f"bv{l}"] = np.broadcast_to(bv.astype(np.float32), (128, C)).copy()
        out[f"boa{l}"] = boa.astype(np.float32).reshape(C, 1).copy()
        names += [f"Wq{l}", f"Wk{l}", f"Wv{l}", f"Wo{l}",
                  f"bq{l}", f"bk{l}", f"bv{l}", f"boa{l}"]
    Wlp = np.asarray(inputs["Wlp"], np.float64)
    out["w12"] = np.stack([Wlp[:C, 0], Wlp[C:, 0]], axis=1).astype(np.float16)
    names.append("w12")
    out["names"] = names
    out["blp"] = float(np.asarray(inputs["blp"]).reshape(-1)[0])
    out["xscale"] = 1.0 - a_s[1]
    return out


# ------------------------------------------------------------------- program

def _build_program(meta, asig1, asig2):
    NPC, BPC, NPAD = meta["NPC"], meta["BPC"], meta["NPAD"]
    T_b, tiles_total = meta["T_b"], meta["tiles_total"]
    Tmax = max(T_b)
    col = np.concatenate([[0], np.cumsum(T_b)]).astype(int)
    C = meta["C"]
    kap = (1.0 - asig2, 1.0)  # skip-add scale on stored input, per layer

    nc = bacc.Bacc("TRN2", target_bir_lowering=False, debug=False,
                   num_devices=CORES)

    # --- I/O -------------------------------------------------------------
    xT_in = nc.dram_tensor("xT", [C, NPC], F16, kind="ExternalInput").ap()
    ekv_in = nc.dram_tensor("ekv", [128, tiles_total], I32,
                            kind="ExternalInput").ap()
    eslot_in = nc.dram_tensor("eslot", [128, tiles_total], BF16,
                              kind="ExternalInput").ap()
    s2_in = nc.dram_tensor("s2", [128, tiles_total * 128], BF16,
                           kind="ExternalInput").ap()
    wspec = {}
    for l in (1, 2):
        for n in ("Wq", "Wk", "Wv", "Wo"):
            wspec[f"{n}{l}"] = ([128, C], F16)
        for n in ("bq", "bk", "bv"):
            wspec[f"{n}{l}"] = ([128, C], F32)
        wspec[f"boa{l}"] = ([128, 1], F32)
    wspec["w12"] = ([128, 2], F16)
    w_in = {n: nc.dram_tensor(n, s, d, kind="ExternalInput").ap()
            for n, (s, d) in wspec.items()}
    uv_out = nc.dram_tensor("uv_out", [2, NPC], F32, kind="ExternalOutput").ap()

    # --- DRAM scratch ----------------------------------------------------
    kv_shard = [nc.dram_tensor(f"kv_shard{l}", [NPC, 2 * C], F16,
                               kind="Internal").ap() for l in (0, 1)]
    kv_full = [nc.dram_tensor(f"kv_full{l}", [NPAD, 2 * C], F16,
                              kind="Internal").ap() for l in (0, 1)]

    with tile.TileContext(nc) as tc, ExitStack() as ctx:
        cpool = ctx.enter_context(tc.tile_pool(name="const", bufs=1))
        sb = ctx.enter_context(tc.tile_pool(name="sb", bufs=2))
        psum = ctx.enter_context(tc.tile_pool(name="ps", bufs=2, space="PSUM"))

        # --- constants into SBUF ----------------------------------------
        W = {}
        for n, (s, d) in wspec.items():
            W[n] = cpool.tile(s, d, tag=f"w_{n}", name=f"wt_{n}")
            nc.sync.dma_start(W[n][:], w_in[n][:])
        ekv_sb = cpool.tile([128, tiles_total], I32, tag="ekv")
        nc.sync.dma_start(ekv_sb[:], ekv_in[:])
        eslot_sb = cpool.tile([128, tiles_total], BF16, tag="eslot")
        nc.sync.dma_start(eslot_sb[:], eslot_in[:])

        ident = cpool.tile([128, 128], F32, tag="ident")
        make_identity(nc, ident[:])
        iota_i = cpool.tile([128, Tmax * 128], I32, tag="iota_i")
        nc.gpsimd.iota(iota_i[:], pattern=[[0, Tmax], [1, 128]], base=0,
                       channel_multiplier=0)
        iota_bf = cpool.tile([128, Tmax * 128], BF16, tag="iota_bf")
        nc.vector.tensor_copy(iota_bf[:], iota_i[:])

        xT_all = cpool.tile([128, NPC], F16, tag="xT_all")
        nc.sync.dma_start(xT_all[:], xT_in[:])
        h1T_all = cpool.tile([128, NPC], F16, tag="h1T")
        aggn_all = [cpool.tile([128, NPC], BF16, tag=f"aggn{l}", name=f"aggn{l}")
                    for l in (0, 1)]
        uv_all = cpool.tile([2, NPC], F32, tag="uv")
        q_all = [cpool.tile([128, NPC], F16, tag=f"q_all{l}", name=f"q_all{l}")
                 for l in (0, 1)]

        def layer(li, srcT_all):
            l = li + 1
            kvs_d, kvf = kv_shard[li], kv_full[li]
            qa = q_all[li]
            # ---- projections for own shard ----
            for b in range(BPC):
                sl = slice(b * 128, (b + 1) * 128)
                lhs = srcT_all[:, sl]
                q_ps = psum.tile([128, C], F32, tag="mm128")
                nc.tensor.matmul(out=q_ps[:], lhsT=lhs, rhs=W[f"Wq{l}"][:],
                                 start=True, stop=True)
                nc.vector.tensor_tensor(out=qa[:, sl], in0=q_ps[:],
                                        in1=W[f"bq{l}"][:], op=OP.add)
                kvs = sb.tile([128, 2 * C], F16, tag="kvs")
                k_ps = psum.tile([128, C], F32, tag="mm128")
                nc.tensor.matmul(out=k_ps[:], lhsT=lhs, rhs=W[f"Wk{l}"][:],
                                 start=True, stop=True)
                nc.vector.tensor_tensor(out=kvs[:, 0:C], in0=k_ps[:],
                                        in1=W[f"bk{l}"][:], op=OP.add)
                v_ps = psum.tile([128, C], F32, tag="mm128")
                nc.tensor.matmul(out=v_ps[:], lhsT=lhs, rhs=W[f"Wv{l}"][:],
                                 start=True, stop=True)
                nc.vector.tensor_tensor(out=kvs[:, C:2 * C], in0=v_ps[:],
                                        in1=W[f"bv{l}"][:], op=OP.add)
                nc.sync.dma_start(kvs_d[sl, :], kvs[:])
            # ---- exchange k/v ----
            nc.gpsimd.collective_compute(
                "AllGather", OP.bypass,
                replica_groups=[list(range(CORES))],
                ins=[kvs_d[:]], outs=[kvf[:]])
            # ---- edge phase ----
            for b in range(BPC):
                T = T_b[b]
                c0 = int(col[b])
                kvg = sb.tile([128, Tmax * 2 * C], F16, tag="kvg")
                for t in range(T):
                    nc.gpsimd.indirect_dma_start(
                        out=kvg[:, t * 256:(t + 1) * 256], out_offset=None,
                        in_=kvf,
                        in_offset=bass.IndirectOffsetOnAxis(
                            ap=ekv_sb[:, c0 + t:c0 + t + 1], axis=0))
                S2 = sb.tile([128, Tmax * 128], BF16, tag="S2")
                nc.sync.dma_start(
                    S2[:, :T * 128],
                    s2_in[:, c0 * 128:(c0 + T) * 128])
                qg = sb.tile([128, Tmax * C], F16, tag="qg")
                for t in range(T):
                    qg_ps = psum.tile([128, C], F32, tag="qg")
                    nc.tensor.matmul(out=qg_ps[:],
                                     lhsT=S2[:, t * 128:(t + 1) * 128],
                                     rhs=qa[:, b * 128:(b + 1) * 128],
                                     start=True, stop=True)
                    nc.vector.tensor_copy(qg[:, t * 128:(t + 1) * 128],
                                          qg_ps[:])
                S = sb.tile([128, Tmax * 128], BF16, tag="S")
                nc.vector.tensor_tensor(
                    out=_apn(S[:], [[128, T], [1, 128]]),
                    in0=_apn(iota_bf[:], [[128, T], [1, 128]]),
                    in1=_expand_last(eslot_sb[:, c0:c0 + T], 128),
                    op=OP.is_equal)
                prod = sb.tile([128, Tmax * C], F16, tag="prod")
                nc.vector.tensor_tensor(
                    out=_apn(prod[:], [[128, T], [1, 128]]),
                    in0=_apn(kvg[:], [[256, T], [1, 128]]),
                    in1=_apn(qg[:], [[128, T], [1, 128]]), op=OP.mult)
                alpha = sb.tile([128, Tmax * 4], F32, tag="alpha")
                nc.vector.tensor_reduce(
                    out=alpha[:, :T * 4],
                    in_=_apn(prod[:], [[32, T * 4], [1, 32]]),
                    axis=mybir.AxisListType.X, op=OP.add)
                ex = sb.tile([128, Tmax * 4], BF16, tag="ex")
                nc.scalar.activation(ex[:, :T * 4], alpha[:, :T * 4], AF.Exp)
                evex = sb.tile([128, Tmax * 132], BF16, tag="evex")
                nc.scalar.activation(_apn(evex[:], [[132, T], [1, 4]]),
                                     alpha[:, :T * 4], AF.Exp)
                nc.vector.tensor_tensor(
                    out=_apn(evex[:], [[132, T], [32, 4], [1, 32]], off=4),
                    in0=_apn(kvg[:], [[256, T], [32, 4], [1, 32]], off=128),
                    in1=_apn(ex[:], [[4, T], [1, 4], [0, 32]]), op=OP.mult)
                agg_ps = psum.tile([128, 132], F32, tag="mm132")
                for t in range(T):
                    nc.tensor.matmul(out=agg_ps[:],
                                     lhsT=S[:, t * 128:(t + 1) * 128],
                                     rhs=evex[:, t * 132:(t + 1) * 132],
                                     start=(t == 0), stop=(t == T - 1))
                den = sb.tile([128, 4], F32, tag="den")
                nc.vector.tensor_scalar_add(den[:], agg_ps[:, 0:4], EPS)
                rd = sb.tile([128, 4], F32, tag="rd")
                nc.vector.reciprocal(rd[:], den[:])
                nc.vector.tensor_tensor(
                    out=_apn(aggn_all[li][:], [[32, 4], [1, 32]], off=b * 128),
                    in0=_apn(agg_ps[:], [[32, 4], [1, 32]], off=4),
                    in1=_expand_last(rd[:], 32), op=OP.mult)
            # ---- pass 2: gelu, output projection, skip (+ decode) ----
            for b in range(BPC):
                sl = slice(b * 128, (b + 1) * 128)
                g = sb.tile([128, C], F32, tag="g")
                nc.scalar.activation(g[:], aggn_all[li][:, sl], AF.Gelu)
                gT_ps = psum.tile([128, C], F32, tag="tr")
                nc.tensor.transpose(out=gT_ps[:], in_=g[:], identity=ident[:])
                gT = sb.tile([128, C], BF16, tag="gTs")
                nc.vector.tensor_copy(gT[:], gT_ps[:])
                hm_ps = psum.tile([128, C], F32, tag="mm128")
                nc.tensor.matmul(out=hm_ps[:], lhsT=W[f"Wo{l}"][:], rhs=gT[:],
                                 start=True, stop=True)
                if l == 1:
                    zt_ap = h1T_all[:, sl]
                else:
                    zt = sb.tile([128, C], F16, tag="zt")
                    zt_ap = zt[:]
                nc.vector.scalar_tensor_tensor(
                    out=zt_ap, in0=srcT_all[:, sl], scalar=kap[li],
                    in1=hm_ps[:], op0=OP.mult, op1=OP.add)
                nc.vector.tensor_tensor(
                    out=zt_ap, in0=zt_ap,
                    in1=_apn(W[f"boa{l}"][:], [[0, 128]]), op=OP.add)
                if l == 2:
                    uv_ps = psum.tile([128, C], F32, tag="mm128")
                    nc.tensor.matmul(out=uv_ps[0:2, :], lhsT=W["w12"][:],
                                     rhs=zt_ap, start=True, stop=True)
                    nc.vector.tensor_copy(uv_all[:, sl], uv_ps[0:2, :])

        layer(0, xT_all)
        layer(1, h1T_all)
        nc.sync.dma_start(uv_out, uv_all[:])

    nc.compile()
    return nc


_CACHE = {}


def _get_program(meta, asig1, asig2, blp):
    key = (meta["N"], meta["E"], meta["T_b"], asig1, asig2)
    if key not in _CACHE:
        _CACHE[key] = _build_program(meta, asig1, asig2)
    return _CACHE[key]


def make_in_maps(inputs):
    inputs = {k: np.asarray(v) for k, v in inputs.items()}
    meta, arrays = _host_prep(np.asarray(inputs["x"], np.float32),
                              inputs["edge_index"])
    w = _prep_weights(inputs)
    N, C, NPC = meta["N"], meta["C"], meta["NPC"]
    xpad = np.zeros((meta["NPAD"], C), dtype=np.float64)
    xpad[:N] = np.asarray(inputs["x"], np.float64)
    xT_full = np.ascontiguousarray((xpad * w["xscale"]).T).astype(np.float16)
    in_maps = []
    for c in range(CORES):
        m = dict(xT=np.ascontiguousarray(xT_full[:, c * NPC:(c + 1) * NPC]),
                 ekv=arrays["ekv"][c], eslot=arrays["eslot"][c],
                 s2=arrays["s2"][c])
        for n in w["names"]:
            m[n] = w[n]
        in_maps.append(m)
    return meta, w, in_maps


def assemble(meta, results, inputs, blp):
    u = np.concatenate([results[c]["uv_out"] for c in range(CORES)], axis=1)
    u1, u2 = u[0], u[1]
    pe, ne = inputs["pos_edge_index"], inputs["neg_edge_index"]
    pos = u1[pe[0]] + u2[pe[1]] + np.float32(blp)
    neg = u1[ne[0]] + u2[ne[1]] + np.float32(blp)
    return pos.astype(np.float32), neg.astype(np.float32)


def kernel(**inputs):
    meta, w, in_maps = make_in_maps(inputs)
    nc = _get_program(meta, w["asig1"], w["asig2"], w["blp"])
    res = bass_utils.run_bass_kernel_spmd(nc, in_maps,
                                          core_ids=list(range(CORES)))
    return assemble(meta, res.results, inputs, w["blp"])


# revision 17
# speedup vs baseline: 2.7084x; 2.7084x over previous
"""HGT link predictor on 8 Trainium2 NeuronCores (Bass/Tile SPMD kernel).

Strategy (hardcoded for nn_HGTLinkPredictor, N=50000 E=800000 P=100000 C=128 H=4 D=32):
 - Shard dst nodes (and their incoming edges) across 8 cores in contiguous
   128-node blocks. Edges sorted by dst on host.
 - Per layer: each core computes q/k/v projections for its node shard from a
   host-transposed feature tile (relation transforms, attention scale and the
   sigmoid-skip coefficients are all folded into the weights on host), writes
   packed [k|v] fp16 rows, AllGathers them so every core can fetch k/v of any
   src node with a single 512B-descriptor indirect DMA per edge (one packed
   row instead of separate k/v/q fetches).
 - q[dst] rows never leave SBUF: per 128-edge tile a one-hot matmul
   (S2^T @ Q_block) on the tensor engine expands the block's q rows to edge
   slots, with the transposed one-hot streamed from a host-precomputed table.
 - Attention logits via fp16 elementwise mult + segmented reduce; exp on the
   scalar engine into a fused [ex | v*ex] bf16 tile so a single 132-column
   matmul per 128-edge tile accumulates both the softmax denominator and the
   weighted value sum in PSUM.
 - Gelu + output projection run as a deferred second pass (one activation
   table load), producing transposed features directly so layer-2 projections
   need no PE transposes. Link decode is one tiny matmul per block; final
   per-edge logits are assembled host-side from per-node partial sums.
"""

import math
import numpy as np
import ml_dtypes
from contextlib import ExitStack

import concourse.bass as bass
import concourse.tile as tile
from concourse import bacc, mybir
from concourse import bass_utils
from concourse.masks import make_identity

F32 = mybir.dt.float32
F16 = mybir.dt.float16
BF16 = mybir.dt.bfloat16
I32 = mybir.dt.int32
AF = mybir.ActivationFunctionType
OP = mybir.AluOpType

CORES = 8
EPS = 1e-30
def _expand_last(ap, n):
    """Append a step-0 (broadcast) innermost dim of size n to an AP."""
    new = [list(p) for p in ap.ap] + [[0, n]]
    return bass.AP(ap.tensor, ap.offset, new)


def _apn(ap, dims, off=0):
    """AP keeping the partition dim of `ap` but custom free-dim pattern."""
    return bass.AP(ap.tensor, ap.offset + off,
                   [list(ap.ap[0])] + [list(d) for d in dims])


# ----------------------------------------------------------------- host prep

def _host_prep(x, edge_index):
    N, C = x.shape
    E = edge_index.shape[1]

    NPC = int(math.ceil(N / (CORES * 128))) * 128   # nodes per core (padded)
    BPC = NPC // 128                                # blocks per core
    NPAD = NPC * CORES

    src = edge_index[0].astype(np.int64)
    dst = edge_index[1].astype(np.int64)
    order = np.argsort(dst, kind="stable")
    s_src, s_dst = src[order], dst[order]

    core_of = s_dst // NPC
    blk_of = (s_dst % NPC) // 128
    gblk = core_of * BPC + blk_of

    cnt = np.zeros((CORES, BPC), dtype=np.int64)
    np.add.at(cnt, (core_of, blk_of), 1)
    T_b = np.maximum(1, np.ceil(cnt.max(axis=0) / 128).astype(np.int64))
    tiles_total = int(T_b.sum())

    blk_starts = np.concatenate([[0], np.cumsum(T_b)])[:-1] * 128
    grp_start = np.zeros(CORES * BPC + 1, dtype=np.int64)
    np.add.at(grp_start, gblk + 1, 1)
    grp_start = np.cumsum(grp_start)
    pos_in_grp = np.arange(E) - grp_start[gblk]

    cap = tiles_total * 128
    ekv = np.zeros((CORES, cap), dtype=np.int32)     # global src node id
    eslot = np.full((CORES, cap), -1.0, dtype=np.float32)

    flat_pos = blk_starts[blk_of] + pos_in_grp
    ekv[core_of, flat_pos] = s_src.astype(np.int32)
    eslot[core_of, flat_pos] = (s_dst % 128).astype(np.float32)

    # transposed one-hot: S2[j, t*128+e] = (eslot[t*128+e] == j), bf16
    s2 = (eslot.reshape(CORES, 1, cap)
          == np.arange(128, dtype=np.float32)[None, :, None])
    s2 = s2.astype(ml_dtypes.bfloat16)
    # [128, tiles_total] partition-major: entry [p, t] = edge t*128+p
    ekv = ekv.reshape(CORES, tiles_total, 128).transpose(0, 2, 1).copy()
    eslot = np.ascontiguousarray(
        eslot.reshape(CORES, tiles_total, 128).transpose(0, 2, 1)
    ).astype(ml_dtypes.bfloat16)

    meta = dict(N=N, C=C, E=E, NPC=NPC, BPC=BPC, NPAD=NPAD,
                T_b=tuple(int(t) for t in T_b), tiles_total=tiles_total)
    arrays = dict(ekv=ekv, eslot=eslot, s2=s2)
    return meta, arrays


def _prep_weights(inputs):
    """Fold relation transforms, attention scale and skip gates into weights.

    Stored features are pre-scaled: x_stored = (1-a1)*x, h1_stored = (1-a2)*h1,
    so the skip connection becomes a plain add and the projection weights are
    divided by the input scale.
    """
    C = inputs["W1k"].shape[0]
    H, D = inputs["a1"].shape[0], inputs["a1"].shape[1]
    a_s = {l: float(1.0 / (1.0 + np.exp(-float(np.asarray(inputs[f"skip{l}"])))))
           for l in (1, 2)}
    out = {"asig1": a_s[1], "asig2": a_s[2]}
    names = []
    for l in (1, 2):
        a_rel = np.asarray(inputs[f"a{l}"], np.float64)
        m_rel = np.asarray(inputs[f"m{l}"], np.float64)
        p_rel = np.asarray(inputs[f"p{l}"], np.float64)
        A = np.zeros((C, C)); M = np.zeros((C, C))
        for h in range(H):
            A[h * D:(h + 1) * D, h * D:(h + 1) * D] = a_rel[h]
            M[h * D:(h + 1) * D, h * D:(h + 1) * D] = m_rel[h]
        qscale = np.repeat(p_rel / np.sqrt(D), D)
        in_scale = 1.0 - a_s[l]
        Wq = np.asarray(inputs[f"W{l}q"], np.float64) * qscale / in_scale
        Wk = np.asarray(inputs[f"W{l}k"], np.float64) @ A / in_scale
        Wv = np.asarray(inputs[f"W{l}v"], np.float64) @ M / in_scale
        bq = np.asarray(inputs[f"b{l}q"], np.float64) * qscale
        bk = np.asarray(inputs[f"b{l}k"], np.float64) @ A
        bv = np.asarray(inputs[f"b{l}v"], np.float64) @ M
        out_scale = a_s[1] * (1.0 - a_s[2]) if l == 1 else a_s[2]
        Wo = np.asarray(inputs[f"Wo{l}"], np.float64) * out_scale
        boa = np.asarray(inputs[f"bo{l}"], np.float64) * out_scale
        out[f"Wq{l}"] = Wq.astype(np.float16)
        out[f"Wk{l}"] = Wk.astype(np.float16)
        out[f"Wv{l}"] = Wv.astype(np.float16)
        out[f"Wo{l}"] = Wo.astype(np.float16)
        out[f"bq{l}"] = np.broadcast_to(bq.astype(np.float32), (128, C)).copy()
        out[f"bk{l}"] = np.broadcast_to(bk.astype(np.float32), (128, C)).copy()
        out[f"bv{l}"] = np.broadcast_to(bv.astype(np.float32), (128, C)).copy()
        out[f"boa{l}"] = boa.astype(np.float32).reshape(C, 1).copy()
        names += [f"Wq{l}", f"Wk{l}", f"Wv{l}", f"Wo{l}",
                  f"bq{l}", f"bk{l}", f"bv{l}", f"boa{l}"]
    Wlp = np.asarray(inputs["Wlp"], np.float64)
    out["w12"] = np.stack([Wlp[:C, 0], Wlp[C:, 0]], axis=1).astype(np.float16)
    names.append("w12")
    out["names"] = names
    out["blp"] = float(np.asarray(inputs["blp"]).reshape(-1)[0])
    out["xscale"] = 1.0 - a_s[1]
    return out


# ------------------------------------------------------------------- program

def _build_program(meta, asig1, asig2):
    NPC, BPC, NPAD = meta["NPC"], meta["BPC"], meta["NPAD"]
    T_b, tiles_total = meta["T_b"], meta["tiles_total"]
    Tmax = max(T_b)
    col = np.concatenate([[0], np.cumsum(T_b)]).astype(int)
    C = meta["C"]
    kap = (1.0 - asig2, 1.0)  # skip-add scale on stored input, per layer

    nc = bacc.Bacc("TRN2", target_bir_lowering=False, debug=False,
                   num_devices=CORES)

    # --- I/O -------------------------------------------------------------
    xT_in = nc.dram_tensor("xT", [C, NPC], F16, kind="ExternalInput").ap()
    ekv_in = nc.dram_tensor("ekv", [128, tiles_total], I32,
                            kind="ExternalInput").ap()
    eslot_in = nc.dram_tensor("eslot", [128, tiles_total], BF16,
                              kind="ExternalInput").ap()
    s2_in = nc.dram_tensor("s2", [128, tiles_total * 128], BF16,
                           kind="ExternalInput").ap()
    wspec = {}
    for l in (1, 2):
        for n in ("Wq", "Wk", "Wv", "Wo"):
            wspec[f"{n}{l}"] = ([128, C], F16)
        for n in ("bq", "bk", "bv"):
            wspec[f"{n}{l}"] = ([128, C], F32)
        wspec[f"boa{l}"] = ([128, 1], F32)
    wspec["w12"] = ([128, 2], F16)
    w_in = {n: nc.dram_tensor(n, s, d, kind="ExternalInput").ap()
            for n, (s, d) in wspec.items()}
    uv_out = nc.dram_tensor("uv_out", [2, NPC], F32, kind="ExternalOutput").ap()

    # --- DRAM scratch ----------------------------------------------------
    kv_shard = [nc.dram_tensor(f"kv_shard{l}", [NPC, 2 * C], F16,
                               kind="Internal").ap() for l in (0, 1)]
    kv_full = [nc.dram_tensor(f"kv_full{l}", [NPAD, 2 * C], F16,
                              kind="Internal").ap() for l in (0, 1)]

    with tile.TileContext(nc) as tc, ExitStack() as ctx:
        cpool = ctx.enter_context(tc.tile_pool(name="const", bufs=1))
        sb = ctx.enter_context(tc.tile_pool(name="sb", bufs=2))
        sbg = ctx.enter_context(tc.tile_pool(name="sbg", bufs=4))
        psum = ctx.enter_context(tc.tile_pool(name="ps", bufs=2, space="PSUM"))

        # --- constants into SBUF ----------------------------------------
        W = {}
        for n, (s, d) in wspec.items():
            W[n] = cpool.tile(s, d, tag=f"w_{n}", name=f"wt_{n}")
            nc.sync.dma_start(W[n][:], w_in[n][:])
        ekv_sb = cpool.tile([128, tiles_total], I32, tag="ekv")
        nc.sync.dma_start(ekv_sb[:], ekv_in[:])
        eslot_sb = cpool.tile([128, tiles_total], BF16, tag="eslot")
        nc.sync.dma_start(eslot_sb[:], eslot_in[:])

        ident = cpool.tile([128, 128], F32, tag="ident")
        make_identity(nc, ident[:])
        iota_i = cpool.tile([128, Tmax * 128], I32, tag="iota_i")
        nc.gpsimd.iota(iota_i[:], pattern=[[0, Tmax], [1, 128]], base=0,
                       channel_multiplier=0)
        iota_bf = cpool.tile([128, Tmax * 128], BF16, tag="iota_bf")
        nc.vector.tensor_copy(iota_bf[:], iota_i[:])

        xT_all = cpool.tile([128, NPC], F16, tag="xT_all")
        nc.sync.dma_start(xT_all[:], xT_in[:])
        h1T_all = cpool.tile([128, NPC], F16, tag="h1T")
        aggn_all = [cpool.tile([128, NPC], BF16, tag=f"aggn{l}", name=f"aggn{l}")
                    for l in (0, 1)]
        uv_all = cpool.tile([2, NPC], F32, tag="uv")
        q_all = [cpool.tile([128, NPC], F16, tag=f"q_all{l}", name=f"q_all{l}")
                 for l in (0, 1)]

        srcT = [xT_all, h1T_all]

        def proj_block(li, b):
            l = li + 1
            kvs_d, qa = kv_shard[li], q_all[li]
            sl = slice(b * 128, (b + 1) * 128)
            lhs = srcT[li][:, sl]
            if True:
                q_ps = psum.tile([128, C], F32, tag="mm128")
                nc.tensor.matmul(out=q_ps[:], lhsT=lhs, rhs=W[f"Wq{l}"][:],
                                 start=True, stop=True)
                nc.vector.tensor_tensor(out=qa[:, sl], in0=q_ps[:],
                                        in1=W[f"bq{l}"][:], op=OP.add)
                kvs = sb.tile([128, 2 * C], F16, tag="kvs")
                k_ps = psum.tile([128, C], F32, tag="mm128")
                nc.tensor.matmul(out=k_ps[:], lhsT=lhs, rhs=W[f"Wk{l}"][:],
                                 start=True, stop=True)
                nc.vector.tensor_tensor(out=kvs[:, 0:C], in0=k_ps[:],
                                        in1=W[f"bk{l}"][:], op=OP.add)
                v_ps = psum.tile([128, C], F32, tag="mm128")
                nc.tensor.matmul(out=v_ps[:], lhsT=lhs, rhs=W[f"Wv{l}"][:],
                                 start=True, stop=True)
                nc.vector.tensor_tensor(out=kvs[:, C:2 * C], in0=v_ps[:],
                                        in1=W[f"bv{l}"][:], op=OP.add)
                nc.sync.dma_start(kvs_d[sl, :], kvs[:])

        def allgather(li):
            nc.gpsimd.collective_compute(
                "AllGather", OP.bypass,
                replica_groups=[list(range(CORES))],
                ins=[kv_shard[li][:]], outs=[kv_full[li][:]])

        def edge_block(li, b):
            l = li + 1
            kvf, qa = kv_full[li], q_all[li]
            if True:
                T = T_b[b]
                c0 = int(col[b])
                kvg = sbg.tile([128, Tmax * 2 * C], F16, tag="kvg")
                for t in range(T):
                    nc.gpsimd.indirect_dma_start(
                        out=kvg[:, t * 256:(t + 1) * 256], out_offset=None,
                        in_=kvf,
                        in_offset=bass.IndirectOffsetOnAxis(
                            ap=ekv_sb[:, c0 + t:c0 + t + 1], axis=0))
                S2 = sb.tile([128, Tmax * 128], BF16, tag="S2")
                nc.sync.dma_start(
                    S2[:, :T * 128],
                    s2_in[:, c0 * 128:(c0 + T) * 128])
                qg = sb.tile([128, Tmax * C], F16, tag="qg")
                for t in range(T):
                    qg_ps = psum.tile([128, C], F32, tag="qg")
                    nc.tensor.matmul(out=qg_ps[:],
                                     lhsT=S2[:, t * 128:(t + 1) * 128],
                                     rhs=qa[:, b * 128:(b + 1) * 128],
                                     start=True, stop=True)
                    nc.vector.tensor_copy(qg[:, t * 128:(t + 1) * 128],
                                          qg_ps[:])
                S = sb.tile([128, Tmax * 128], BF16, tag="S")
                nc.vector.tensor_tensor(
                    out=_apn(S[:], [[128, T], [1, 128]]),
                    in0=_apn(iota_bf[:], [[128, T], [1, 128]]),
                    in1=_expand_last(eslot_sb[:, c0:c0 + T], 128),
                    op=OP.is_equal)
                prod = sb.tile([128, Tmax * C], F16, tag="prod")
                nc.vector.tensor_tensor(
                    out=_apn(prod[:], [[128, T], [1, 128]]),
                    in0=_apn(kvg[:], [[256, T], [1, 128]]),
                    in1=_apn(qg[:], [[128, T], [1, 128]]), op=OP.mult)
                alpha = sb.tile([128, Tmax * 4], F32, tag="alpha")
                nc.vector.tensor_reduce(
                    out=alpha[:, :T * 4],
                    in_=_apn(prod[:], [[32, T * 4], [1, 32]]),
                    axis=mybir.AxisListType.X, op=OP.add)
                ex = sb.tile([128, Tmax * 4], BF16, tag="ex")
                nc.scalar.activation(ex[:, :T * 4], alpha[:, :T * 4], AF.Exp)
                evex = sb.tile([128, Tmax * 132], BF16, tag="evex")
                nc.scalar.activation(_apn(evex[:], [[132, T], [1, 4]]),
                                     alpha[:, :T * 4], AF.Exp)
                nc.vector.tensor_tensor(
                    out=_apn(evex[:], [[132, T], [32, 4], [1, 32]], off=4),
                    in0=_apn(kvg[:], [[256, T], [32, 4], [1, 32]], off=128),
                    in1=_apn(ex[:], [[4, T], [1, 4], [0, 32]]), op=OP.mult)
                agg_ps = psum.tile([128, 132], F32, tag="mm132")
                for t in range(T):
                    nc.tensor.matmul(out=agg_ps[:],
                                     lhsT=S[:, t * 128:(t + 1) * 128],
                                     rhs=evex[:, t * 132:(t + 1) * 132],
                                     start=(t == 0), stop=(t == T - 1))
                den = sb.tile([128, 4], F32, tag="den")
                nc.vector.tensor_scalar_add(den[:], agg_ps[:, 0:4], EPS)
                rd = sb.tile([128, 4], F32, tag="rd")
                nc.vector.reciprocal(rd[:], den[:])
                nc.vector.tensor_tensor(
                    out=_apn(aggn_all[li][:], [[32, 4], [1, 32]], off=b * 128),
                    in0=_apn(agg_ps[:], [[32, 4], [1, 32]], off=4),
                    in1=_expand_last(rd[:], 32), op=OP.mult)

        def pass2_block(li, b):
            l = li + 1
            if True:
                sl = slice(b * 128, (b + 1) * 128)
                g = sb.tile([128, C], F32, tag="g")
                nc.scalar.activation(g[:], aggn_all[li][:, sl], AF.Gelu)
                gT_ps = psum.tile([128, C], F32, tag="tr")
                nc.tensor.transpose(out=gT_ps[:], in_=g[:], identity=ident[:])
                gT = sb.tile([128, C], BF16, tag="gTs")
                nc.vector.tensor_copy(gT[:], gT_ps[:])
                hm_ps = psum.tile([128, C], F32, tag="mm128")
                nc.tensor.matmul(out=hm_ps[:], lhsT=W[f"Wo{l}"][:], rhs=gT[:],
                                 start=True, stop=True)
                if l == 1:
                    zt_ap = h1T_all[:, sl]
                else:
                    zt = sb.tile([128, C], F16, tag="zt")
                    zt_ap = zt[:]
                nc.vector.scalar_tensor_tensor(
                    out=zt_ap, in0=srcT[li][:, sl], scalar=kap[li],
                    in1=hm_ps[:], op0=OP.mult, op1=OP.add)
                nc.vector.tensor_tensor(
                    out=zt_ap, in0=zt_ap,
                    in1=_apn(W[f"boa{l}"][:], [[0, 128]]), op=OP.add)
                if l == 2:
                    uv_ps = psum.tile([128, C], F32, tag="mm128")
                    nc.tensor.matmul(out=uv_ps[0:2, :], lhsT=W["w12"][:],
                                     rhs=zt_ap, start=True, stop=True)
                    nc.vector.tensor_copy(uv_all[:, sl], uv_ps[0:2, :])

        # interleaved schedule: layer-1 pass-2 and layer-2 projections ride
        # inside the layer-1 edge loop so AllGather 2 fires with minimal tail.
        for b in range(BPC):
            proj_block(0, b)
        allgather(0)
        for b in range(BPC):
            edge_block(0, b)
            pass2_block(0, b)
            proj_block(1, b)
        allgather(1)
        for b in range(BPC):
            edge_block(1, b)
            pass2_block(1, b)
        nc.sync.dma_start(uv_out, uv_all[:])

    nc.compile()
    return nc


_CACHE = {}


def _get_program(meta, asig1, asig2, blp):
    key = (meta["N"], meta["E"], meta["T_b"], asig1, asig2)
    if key not in _CACHE:
        _CACHE[key] = _build_program(meta, asig1, asig2)
    return _CACHE[key]


def make_in_maps(inputs):
    inputs = {k: np.asarray(v) for k, v in inputs.items()}
    meta, arrays = _host_prep(np.asarray(inputs["x"], np.float32),
                              inputs["edge_index"])
    w = _prep_weights(inputs)
    N, C, NPC = meta["N"], meta["C"], meta["NPC"]
    xpad = np.zeros((meta["NPAD"], C), dtype=np.float64)
    xpad[:N] = np.asarray(inputs["x"], np.float64)
    xT_full = np.ascontiguousarray((xpad * w["xscale"]).T).astype(np.float16)
    in_maps = []
    for c in range(CORES):
        m = dict(xT=np.ascontiguousarray(xT_full[:, c * NPC:(c + 1) * NPC]),
                 ekv=arrays["ekv"][c], eslot=arrays["eslot"][c],
                 s2=arrays["s2"][c])
        for n in w["names"]:
            m[n] = w[n]
        in_maps.append(m)
    return meta, w, in_maps


def assemble(meta, results, inputs, blp):
    u = np.concatenate([results[c]["uv_out"] for c in range(CORES)], axis=1)
    u1, u2 = u[0], u[1]
    pe, ne = inputs["pos_edge_index"], inputs["neg_edge_index"]
    pos = u1[pe[0]] + u2[pe[1]] + np.float32(blp)
    neg = u1[ne[0]] + u2[ne[1]] + np.float32(blp)
    return pos.astype(np.float32), neg.astype(np.float32)


def kernel(**inputs):
    meta, w, in_maps = make_in_maps(inputs)
    nc = _get_program(meta, w["asig1"], w["asig2"], w["blp"])
    res = bass_utils.run_bass_kernel_spmd(nc, in_maps,
                                          core_ids=list(range(CORES)))
    return assemble(meta, res.results, inputs, w["blp"])


# revision 19
# speedup vs baseline: 2.7816x; 1.0270x over previous
"""HGT link predictor on 8 Trainium2 NeuronCores (Bass/Tile SPMD kernel).

Strategy (hardcoded for nn_HGTLinkPredictor, N=50000 E=800000 P=100000 C=128 H=4 D=32):
 - Shard dst nodes (and their incoming edges) across 8 cores in contiguous
   128-node blocks. Edges sorted by dst on host.
 - Per layer: each core computes q/k/v projections for its node shard from a
   host-transposed feature tile (relation transforms, attention scale and the
   sigmoid-skip coefficients are all folded into the weights on host), writes
   packed [k|v] fp16 rows, AllGathers them so every core can fetch k/v of any
   src node with a single 512B-descriptor indirect DMA per edge (one packed
   row instead of separate k/v/q fetches).
 - q[dst] rows never leave SBUF: per 128-edge tile a one-hot matmul
   (S2^T @ Q_block) on the tensor engine expands the block's q rows to edge
   slots, with the transposed one-hot streamed from a host-precomputed table.
 - Attention logits via fp16 elementwise mult + segmented reduce; exp on the
   scalar engine into a fused [ex | v*ex] bf16 tile so a single 132-column
   matmul per 128-edge tile accumulates both the softmax denominator and the
   weighted value sum in PSUM.
 - Gelu + output projection run as a deferred second pass (one activation
   table load), producing transposed features directly so layer-2 projections
   need no PE transposes. Link decode is one tiny matmul per block; final
   per-edge logits are assembled host-side from per-node partial sums.
"""

import math
import numpy as np
import ml_dtypes
from contextlib import ExitStack

import concourse.bass as bass
import concourse.tile as tile
from concourse import bacc, mybir
from concourse import bass_utils
from concourse.masks import make_identity

F32 = mybir.dt.float32
F16 = mybir.dt.float16
BF16 = mybir.dt.bfloat16
I32 = mybir.dt.int32
AF = mybir.ActivationFunctionType
OP = mybir.AluOpType

CORES = 8
EPS = 1e-30
def _expand_last(ap, n):
    """Append a step-0 (broadcast) innermost dim of size n to an AP."""
    new = [list(p) for p in ap.ap] + [[0, n]]
    return bass.AP(ap.tensor, ap.offset, new)


def _apn(ap, dims, off=0):
    """AP keeping the partition dim of `ap` but custom free-dim pattern."""
    return bass.AP(ap.tensor, ap.offset + off,
                   [list(ap.ap[0])] + [list(d) for d in dims])


# ----------------------------------------------------------------- host prep

def _host_prep(x, edge_index):
    N, C = x.shape
    E = edge_index.shape[1]

    NPC = int(math.ceil(N / (CORES * 128))) * 128   # nodes per core (padded)
    BPC = NPC // 128                                # blocks per core
    NPAD = NPC * CORES

    src = edge_index[0].astype(np.int64)
    dst = edge_index[1].astype(np.int64)
    order = np.argsort(dst, kind="stable")
    s_src, s_dst = src[order], dst[order]

    # Re-pair the NPAD//128 dst blocks into (core, index) groups of similar
    # load: T_b is the max over cores at each index, so grouping blocks of
    # similar edge count minimizes total tiles.
    NBLK = NPAD // 128
    cnt_g = np.bincount(dst // 128, minlength=NBLK)
    order_b = np.argsort(-cnt_g, kind="stable")
    place_core = np.empty(NBLK, np.int64)
    place_idx = np.empty(NBLK, np.int64)
    for g in range(BPC):
        grp = order_b[g * CORES:(g + 1) * CORES]
        place_core[grp] = np.arange(len(grp))
        place_idx[grp] = g
    # global slot row of node n (row in kv_full / uv order)
    slotrow = (place_core[:, None] * NPC + place_idx[:, None] * 128
               + np.arange(128)[None, :]).reshape(-1)        # [NPAD]
    inv_place = np.empty((CORES, BPC), np.int64)
    inv_place[place_core, place_idx] = np.arange(NBLK)

    core_of = place_core[s_dst // 128]
    blk_of = place_idx[s_dst // 128]
    gblk = core_of * BPC + blk_of

    cnt = np.zeros((CORES, BPC), dtype=np.int64)
    np.add.at(cnt, (core_of, blk_of), 1)
    T_b = np.maximum(1, np.ceil(cnt.max(axis=0) / 128).astype(np.int64))
    tiles_total = int(T_b.sum())

    blk_starts = np.concatenate([[0], np.cumsum(T_b)])[:-1] * 128
    # re-pairing makes gblk non-contiguous in dst-sorted order: cumcount
    order2 = np.argsort(gblk, kind="stable")
    ks = gblk[order2]
    grp_start = np.concatenate(
        [[0], np.cumsum(np.bincount(ks, minlength=CORES * BPC))])
    pos_in_grp = np.empty(E, dtype=np.int64)
    pos_in_grp[order2] = np.arange(E) - grp_start[ks]

    cap = tiles_total * 128
    ekv = np.zeros((CORES, cap), dtype=np.int32)     # global src node id
    eslot = np.full((CORES, cap), -1.0, dtype=np.float32)

    flat_pos = blk_starts[blk_of] + pos_in_grp
    ekv[core_of, flat_pos] = slotrow[s_src].astype(np.int32)
    eslot[core_of, flat_pos] = (s_dst % 128).astype(np.float32)

    # transposed one-hot: S2[j, t*128+e] = (eslot[t*128+e] == j), bf16
    s2 = (eslot.reshape(CORES, 1, cap)
          == np.arange(128, dtype=np.float32)[None, :, None])
    s2 = s2.astype(ml_dtypes.bfloat16)
    # [128, tiles_total] partition-major: entry [p, t] = edge t*128+p
    ekv = ekv.reshape(CORES, tiles_total, 128).transpose(0, 2, 1).copy()
    eslot = np.ascontiguousarray(
        eslot.reshape(CORES, tiles_total, 128).transpose(0, 2, 1)
    ).astype(ml_dtypes.bfloat16)

    meta = dict(N=N, C=C, E=E, NPC=NPC, BPC=BPC, NPAD=NPAD,
                T_b=tuple(int(t) for t in T_b), tiles_total=tiles_total)
    arrays = dict(ekv=ekv, eslot=eslot, s2=s2,
                  slotrow=slotrow, inv_place=inv_place)
    return meta, arrays


def _prep_weights(inputs):
    """Fold relation transforms, attention scale and skip gates into weights.

    Stored features are pre-scaled: x_stored = (1-a1)*x, h1_stored = (1-a2)*h1,
    so the skip connection becomes a plain add and the projection weights are
    divided by the input scale.
    """
    C = inputs["W1k"].shape[0]
    H, D = inputs["a1"].shape[0], inputs["a1"].shape[1]
    a_s = {l: float(1.0 / (1.0 + np.exp(-float(np.asarray(inputs[f"skip{l}"])))))
           for l in (1, 2)}
    out = {"asig1": a_s[1], "asig2": a_s[2]}
    names = []
    for l in (1, 2):
        a_rel = np.asarray(inputs[f"a{l}"], np.float64)
        m_rel = np.asarray(inputs[f"m{l}"], np.float64)
        p_rel = np.asarray(inputs[f"p{l}"], np.float64)
        A = np.zeros((C, C)); M = np.zeros((C, C))
        for h in range(H):
            A[h * D:(h + 1) * D, h * D:(h + 1) * D] = a_rel[h]
            M[h * D:(h + 1) * D, h * D:(h + 1) * D] = m_rel[h]
        qscale = np.repeat(p_rel / np.sqrt(D), D)
        in_scale = 1.0 - a_s[l]
        Wq = np.asarray(inputs[f"W{l}q"], np.float64) * qscale / in_scale
        Wk = np.asarray(inputs[f"W{l}k"], np.float64) @ A / in_scale
        Wv = np.asarray(inputs[f"W{l}v"], np.float64) @ M / in_scale
        bq = np.asarray(inputs[f"b{l}q"], np.float64) * qscale
        bk = np.asarray(inputs[f"b{l}k"], np.float64) @ A
        bv = np.asarray(inputs[f"b{l}v"], np.float64) @ M
        out_scale = a_s[1] * (1.0 - a_s[2]) if l == 1 else a_s[2]
        Wo = np.asarray(inputs[f"Wo{l}"], np.float64) * out_scale
        boa = np.asarray(inputs[f"bo{l}"], np.float64) * out_scale
        out[f"Wq{l}"] = Wq.astype(np.float16)
        out[f"Wk{l}"] = Wk.astype(np.float16)
        out[f"Wv{l}"] = Wv.astype(np.float16)
        out[f"Wo{l}"] = Wo.astype(np.float16)
        out[f"bq{l}"] = np.broadcast_to(bq.astype(np.float32), (128, C)).copy()
        out[f"bk{l}"] = np.broadcast_to(bk.astype(np.float32), (128, C)).copy()
        out[f"bv{l}"] = np.broadcast_to(bv.astype(np.float32), (128, C)).copy()
        out[f"boa{l}"] = boa.astype(np.float32).reshape(C, 1).copy()
        names += [f"Wq{l}", f"Wk{l}", f"Wv{l}", f"Wo{l}",
                  f"bq{l}", f"bk{l}", f"bv{l}", f"boa{l}"]
    Wlp = np.asarray(inputs["Wlp"], np.float64)
    out["w12"] = np.stack([Wlp[:C, 0], Wlp[C:, 0]], axis=1).astype(np.float16)
    names.append("w12")
    out["names"] = names
    out["blp"] = float(np.asarray(inputs["blp"]).reshape(-1)[0])
    out["xscale"] = 1.0 - a_s[1]
    return out


# ------------------------------------------------------------------- program

def _build_program(meta, asig1, asig2):
    NPC, BPC, NPAD = meta["NPC"], meta["BPC"], meta["NPAD"]
    T_b, tiles_total = meta["T_b"], meta["tiles_total"]
    Tmax = max(T_b)
    col = np.concatenate([[0], np.cumsum(T_b)]).astype(int)
    C = meta["C"]
    kap = (1.0 - asig2, 1.0)  # skip-add scale on stored input, per layer

    nc = bacc.Bacc("TRN2", target_bir_lowering=False, debug=False,
                   num_devices=CORES)

    # --- I/O -------------------------------------------------------------
    xT_in = nc.dram_tensor("xT", [C, NPC], F16, kind="ExternalInput").ap()
    ekv_in = nc.dram_tensor("ekv", [128, tiles_total], I32,
                            kind="ExternalInput").ap()
    eslot_in = nc.dram_tensor("eslot", [128, tiles_total], BF16,
                              kind="ExternalInput").ap()
    s2_in = nc.dram_tensor("s2", [128, tiles_total * 128], BF16,
                           kind="ExternalInput").ap()
    wspec = {}
    for l in (1, 2):
        for n in ("Wq", "Wk", "Wv", "Wo"):
            wspec[f"{n}{l}"] = ([128, C], F16)
        for n in ("bq", "bk", "bv"):
            wspec[f"{n}{l}"] = ([128, C], F32)
        wspec[f"boa{l}"] = ([128, 1], F32)
    wspec["w12"] = ([128, 2], F16)
    w_in = {n: nc.dram_tensor(n, s, d, kind="ExternalInput").ap()
            for n, (s, d) in wspec.items()}
    uv_out = nc.dram_tensor("uv_out", [2, NPC], F32, kind="ExternalOutput").ap()

    # --- DRAM scratch ----------------------------------------------------
    kv_shard = [nc.dram_tensor(f"kv_shard{l}", [NPC, 2 * C], F16,
                               kind="Internal").ap() for l in (0, 1)]
    kv_full = [nc.dram_tensor(f"kv_full{l}", [NPAD, 2 * C], F16,
                              kind="Internal").ap() for l in (0, 1)]

    with tile.TileContext(nc) as tc, ExitStack() as ctx:
        cpool = ctx.enter_context(tc.tile_pool(name="const", bufs=1))
        sb = ctx.enter_context(tc.tile_pool(name="sb", bufs=2))
        sbg = ctx.enter_context(tc.tile_pool(name="sbg", bufs=4))
        psum = ctx.enter_context(tc.tile_pool(name="ps", bufs=2, space="PSUM"))

        # --- constants into SBUF ----------------------------------------
        W = {}
        for n, (s, d) in wspec.items():
            W[n] = cpool.tile(s, d, tag=f"w_{n}", name=f"wt_{n}")
            nc.sync.dma_start(W[n][:], w_in[n][:])
        ekv_sb = cpool.tile([128, tiles_total], I32, tag="ekv")
        nc.sync.dma_start(ekv_sb[:], ekv_in[:])
        eslot_sb = cpool.tile([128, tiles_total], BF16, tag="eslot")
        nc.sync.dma_start(eslot_sb[:], eslot_in[:])

        ident = cpool.tile([128, 128], F32, tag="ident")
        make_identity(nc, ident[:])
        iota_i = cpool.tile([128, Tmax * 128], I32, tag="iota_i")
        nc.gpsimd.iota(iota_i[:], pattern=[[0, Tmax], [1, 128]], base=0,
                       channel_multiplier=0)
        iota_bf = cpool.tile([128, Tmax * 128], BF16, tag="iota_bf")
        nc.vector.tensor_copy(iota_bf[:], iota_i[:])

        xT_all = cpool.tile([128, NPC], F16, tag="xT_all")
        nc.sync.dma_start(xT_all[:], xT_in[:])
        h1T_all = cpool.tile([128, NPC], F16, tag="h1T")
        aggn_all = [cpool.tile([128, NPC], BF16, tag=f"aggn{l}", name=f"aggn{l}")
                    for l in (0, 1)]
        uv_all = cpool.tile([2, NPC], F32, tag="uv")
        q_all = [cpool.tile([128, NPC], F16, tag=f"q_all{l}", name=f"q_all{l}")
                 for l in (0, 1)]

        srcT = [xT_all, h1T_all]

        def proj_block(li, b):
            l = li + 1
            kvs_d, qa = kv_shard[li], q_all[li]
            sl = slice(b * 128, (b + 1) * 128)
            lhs = srcT[li][:, sl]
            if True:
                q_ps = psum.tile([128, C], F32, tag="mm128")
                nc.tensor.matmul(out=q_ps[:], lhsT=lhs, rhs=W[f"Wq{l}"][:],
                                 start=True, stop=True)
                nc.vector.tensor_tensor(out=qa[:, sl], in0=q_ps[:],
                                        in1=W[f"bq{l}"][:], op=OP.add)
                kvs = sb.tile([128, 2 * C], F16, tag="kvs")
                k_ps = psum.tile([128, C], F32, tag="mm128")
                nc.tensor.matmul(out=k_ps[:], lhsT=lhs, rhs=W[f"Wk{l}"][:],
                                 start=True, stop=True)
                nc.vector.tensor_tensor(out=kvs[:, 0:C], in0=k_ps[:],
                                        in1=W[f"bk{l}"][:], op=OP.add)
                v_ps = psum.tile([128, C], F32, tag="mm128")
                nc.tensor.matmul(out=v_ps[:], lhsT=lhs, rhs=W[f"Wv{l}"][:],
                                 start=True, stop=True)
                nc.vector.tensor_tensor(out=kvs[:, C:2 * C], in0=v_ps[:],
                                        in1=W[f"bv{l}"][:], op=OP.add)
                nc.sync.dma_start(kvs_d[sl, :], kvs[:])

        def allgather(li):
            nc.gpsimd.collective_compute(
                "AllGather", OP.bypass,
                replica_groups=[list(range(CORES))],
                ins=[kv_shard[li][:]], outs=[kv_full[li][:]])

        def edge_block(li, b):
            l = li + 1
            kvf, qa = kv_full[li], q_all[li]
            if True:
                T = T_b[b]
                c0 = int(col[b])
                kvg = sbg.tile([128, Tmax * 2 * C], F16, tag="kvg")
                for t in range(T):
                    nc.gpsimd.indirect_dma_start(
                        out=kvg[:, t * 256:(t + 1) * 256], out_offset=None,
                        in_=kvf,
                        in_offset=bass.IndirectOffsetOnAxis(
                            ap=ekv_sb[:, c0 + t:c0 + t + 1], axis=0))
                S2 = sb.tile([128, Tmax * 128], BF16, tag="S2")
                nc.sync.dma_start(
                    S2[:, :T * 128],
                    s2_in[:, c0 * 128:(c0 + T) * 128])
                qg = sb.tile([128, Tmax * C], F16, tag="qg")
                for t in range(T):
                    qg_ps = psum.tile([128, C], F32, tag="qg")
                    nc.tensor.matmul(out=qg_ps[:],
                                     lhsT=S2[:, t * 128:(t + 1) * 128],
                                     rhs=qa[:, b * 128:(b + 1) * 128],
                                     start=True, stop=True)
                    nc.vector.tensor_copy(qg[:, t * 128:(t + 1) * 128],
                                          qg_ps[:])
                S = sb.tile([128, Tmax * 128], BF16, tag="S")
                nc.vector.tensor_tensor(
                    out=_apn(S[:], [[128, T], [1, 128]]),
                    in0=_apn(iota_bf[:], [[128, T], [1, 128]]),
                    in1=_expand_last(eslot_sb[:, c0:c0 + T], 128),
                    op=OP.is_equal)
                prod = sb.tile([128, Tmax * C], F16, tag="prod")
                nc.vector.tensor_tensor(
                    out=_apn(prod[:], [[128, T], [1, 128]]),
                    in0=_apn(kvg[:], [[256, T], [1, 128]]),
                    in1=_apn(qg[:], [[128, T], [1, 128]]), op=OP.mult)
                alpha = sb.tile([128, Tmax * 4], F32, tag="alpha")
                nc.vector.tensor_reduce(
                    out=alpha[:, :T * 4],
                    in_=_apn(prod[:], [[32, T * 4], [1, 32]]),
                    axis=mybir.AxisListType.X, op=OP.add)
                ex = sb.tile([128, Tmax * 4], BF16, tag="ex")
                nc.scalar.activation(ex[:, :T * 4], alpha[:, :T * 4], AF.Exp)
                evex = sb.tile([128, Tmax * 132], BF16, tag="evex")
                nc.scalar.activation(_apn(evex[:], [[132, T], [1, 4]]),
                                     alpha[:, :T * 4], AF.Exp)
                nc.vector.tensor_tensor(
                    out=_apn(evex[:], [[132, T], [32, 4], [1, 32]], off=4),
                    in0=_apn(kvg[:], [[256, T], [32, 4], [1, 32]], off=128),
                    in1=_apn(ex[:], [[4, T], [1, 4], [0, 32]]), op=OP.mult)
                agg_ps = psum.tile([128, 132], F32, tag="mm132")
                for t in range(T):
                    nc.tensor.matmul(out=agg_ps[:],
                                     lhsT=S[:, t * 128:(t + 1) * 128],
                                     rhs=evex[:, t * 132:(t + 1) * 132],
                                     start=(t == 0), stop=(t == T - 1))
                den = sb.tile([128, 4], F32, tag="den")
                nc.vector.tensor_scalar_add(den[:], agg_ps[:, 0:4], EPS)
                rd = sb.tile([128, 4], F32, tag="rd")
                nc.vector.reciprocal(rd[:], den[:])
                nc.vector.tensor_tensor(
                    out=_apn(aggn_all[li][:], [[32, 4], [1, 32]], off=b * 128),
                    in0=_apn(agg_ps[:], [[32, 4], [1, 32]], off=4),
                    in1=_expand_last(rd[:], 32), op=OP.mult)

        def pass2_block(li, b):
            l = li + 1
            if True:
                sl = slice(b * 128, (b + 1) * 128)
                g = sb.tile([128, C], F32, tag="g")
                nc.scalar.activation(g[:], aggn_all[li][:, sl], AF.Gelu)
                gT_ps = psum.tile([128, C], F32, tag="tr")
                nc.tensor.transpose(out=gT_ps[:], in_=g[:], identity=ident[:])
                gT = sb.tile([128, C], BF16, tag="gTs")
                nc.vector.tensor_copy(gT[:], gT_ps[:])
                hm_ps = psum.tile([128, C], F32, tag="mm128")
                nc.tensor.matmul(out=hm_ps[:], lhsT=W[f"Wo{l}"][:], rhs=gT[:],
                                 start=True, stop=True)
                if l == 1:
                    zt_ap = h1T_all[:, sl]
                else:
                    zt = sb.tile([128, C], F16, tag="zt")
                    zt_ap = zt[:]
                nc.vector.scalar_tensor_tensor(
                    out=zt_ap, in0=srcT[li][:, sl], scalar=kap[li],
                    in1=hm_ps[:], op0=OP.mult, op1=OP.add)
                nc.vector.tensor_tensor(
                    out=zt_ap, in0=zt_ap,
                    in1=_apn(W[f"boa{l}"][:], [[0, 128]]), op=OP.add)
                if l == 2:
                    uv_ps = psum.tile([128, C], F32, tag="mm128")
                    nc.tensor.matmul(out=uv_ps[0:2, :], lhsT=W["w12"][:],
                                     rhs=zt_ap, start=True, stop=True)
                    nc.vector.tensor_copy(uv_all[:, sl], uv_ps[0:2, :])

        # interleaved schedule: layer-1 pass-2 and layer-2 projections ride
        # inside the layer-1 edge loop so AllGather 2 fires with minimal tail.
        for b in range(BPC):
            proj_block(0, b)
        allgather(0)
        for b in range(BPC):
            edge_block(0, b)
            pass2_block(0, b)
            proj_block(1, b)
        allgather(1)
        for b in range(BPC):
            edge_block(1, b)
            pass2_block(1, b)
        nc.sync.dma_start(uv_out, uv_all[:])

    nc.compile()
    return nc


_CACHE = {}


def _get_program(meta, asig1, asig2, blp):
    key = (meta["N"], meta["E"], meta["T_b"], asig1, asig2)
    if key not in _CACHE:
        _CACHE[key] = _build_program(meta, asig1, asig2)
    return _CACHE[key]


def make_in_maps(inputs):
    inputs = {k: np.asarray(v) for k, v in inputs.items()}
    meta, arrays = _host_prep(np.asarray(inputs["x"], np.float32),
                              inputs["edge_index"])
    w = _prep_weights(inputs)
    N, C, NPC = meta["N"], meta["C"], meta["NPC"]
    xpad = np.zeros((meta["NPAD"], C), dtype=np.float64)
    xpad[:N] = np.asarray(inputs["x"], np.float64)
    # column order: global slot s holds node inv_place-block's rows
    inv_place = arrays["inv_place"]
    gorder = (inv_place.reshape(-1)[:, None] * 128
              + np.arange(128)[None, :]).reshape(-1)         # slot -> node
    xT_full = np.ascontiguousarray(
        (xpad * w["xscale"]).T[:, gorder]).astype(np.float16)
    in_maps = []
    for c in range(CORES):
        m = dict(xT=np.ascontiguousarray(xT_full[:, c * NPC:(c + 1) * NPC]),
                 ekv=arrays["ekv"][c], eslot=arrays["eslot"][c],
                 s2=arrays["s2"][c])
        for n in w["names"]:
            m[n] = w[n]
        in_maps.append(m)
    meta["slotrow"] = arrays["slotrow"]
    return meta, w, in_maps


def assemble(meta, results, inputs, blp):
    u = np.concatenate([results[c]["uv_out"] for c in range(CORES)], axis=1)
    sr = meta["slotrow"]
    u1, u2 = u[0][sr], u[1][sr]          # back to node-id order
    pe, ne = inputs["pos_edge_index"], inputs["neg_edge_index"]
    pos = u1[pe[0]] + u2[pe[1]] + np.float32(blp)
    neg = u1[ne[0]] + u2[ne[1]] + np.float32(blp)
    return pos.astype(np.float32), neg.astype(np.float32)


def kernel(**inputs):
    meta, w, in_maps = make_in_maps(inputs)
    nc = _get_program(meta, w["asig1"], w["asig2"], w["blp"])
    res = bass_utils.run_bass_kernel_spmd(nc, in_maps,
                                          core_ids=list(range(CORES)))
    return assemble(meta, res.results, inputs, w["blp"])


# revision 20
# speedup vs baseline: 2.8678x; 1.0310x over previous
"""HGT link predictor on 8 Trainium2 NeuronCores (Bass/Tile SPMD kernel).

Strategy (hardcoded for nn_HGTLinkPredictor, N=50000 E=800000 P=100000 C=128 H=4 D=32):
 - Shard dst nodes (and their incoming edges) across 8 cores in contiguous
   128-node blocks. Edges sorted by dst on host.
 - Per layer: each core computes q/k/v projections for its node shard from a
   host-transposed feature tile (relation transforms, attention scale and the
   sigmoid-skip coefficients are all folded into the weights on host), writes
   packed [k|v] fp16 rows, AllGathers them so every core can fetch k/v of any
   src node with a single 512B-descriptor indirect DMA per edge (one packed
   row instead of separate k/v/q fetches).
 - q[dst] rows never leave SBUF: per 128-edge tile a one-hot matmul
   (S2^T @ Q_block) on the tensor engine expands the block's q rows to edge
   slots, with the transposed one-hot streamed from a host-precomputed table.
 - Attention logits via fp16 elementwise mult + segmented reduce; exp on the
   scalar engine into a fused [ex | v*ex] bf16 tile so a single 132-column
   matmul per 128-edge tile accumulates both the softmax denominator and the
   weighted value sum in PSUM.
 - Gelu + output projection run as a deferred second pass (one activation
   table load), producing transposed features directly so layer-2 projections
   need no PE transposes. Link decode is one tiny matmul per block; final
   per-edge logits are assembled host-side from per-node partial sums.
"""

import math
import numpy as np
import ml_dtypes
from contextlib import ExitStack

import concourse.bass as bass
import concourse.tile as tile
from concourse import bacc, mybir
from concourse import bass_utils
from concourse.masks import make_identity

F32 = mybir.dt.float32
F16 = mybir.dt.float16
BF16 = mybir.dt.bfloat16
I32 = mybir.dt.int32
AF = mybir.ActivationFunctionType
OP = mybir.AluOpType

CORES = 8
EPS = 1e-30
def _expand_last(ap, n):
    """Append a step-0 (broadcast) innermost dim of size n to an AP."""
    new = [list(p) for p in ap.ap] + [[0, n]]
    return bass.AP(ap.tensor, ap.offset, new)


def _apn(ap, dims, off=0):
    """AP keeping the partition dim of `ap` but custom free-dim pattern."""
    return bass.AP(ap.tensor, ap.offset + off,
                   [list(ap.ap[0])] + [list(d) for d in dims])


# ----------------------------------------------------------------- host prep

def _host_prep(x, edge_index):
    N, C = x.shape
    E = edge_index.shape[1]

    NPC = int(math.ceil(N / (CORES * 128))) * 128   # nodes per core (padded)
    BPC = NPC // 128                                # blocks per core
    NPAD = NPC * CORES

    src = edge_index[0].astype(np.int64)
    dst = edge_index[1].astype(np.int64)
    order = np.argsort(dst, kind="stable")
    s_src, s_dst = src[order], dst[order]

    # Balanced dst partition: assign nodes to blocks (exactly 128 each) by
    # greedy degree balancing, then pair blocks of similar load into
    # (core, index) groups — T_b is the max over cores at each index.
    import heapq
    NBLK = NPAD // 128
    deg = np.bincount(dst, minlength=NPAD)
    order_n = np.argsort(-deg, kind="stable")
    heap = [(0, 0, b) for b in range(NBLK)]
    heapq.heapify(heap)
    bin_of = np.empty(NPAD, np.int64)
    slot_in = np.empty(NPAD, np.int64)
    for n in order_n:
        s, cb, b = heapq.heappop(heap)
        bin_of[n] = b
        slot_in[n] = cb
        if cb + 1 < 128:
            heapq.heappush(heap, (s + int(deg[n]), cb + 1, b))
    binsum = np.bincount(bin_of, weights=deg.astype(np.float64),
                         minlength=NBLK)
    order_b = np.argsort(-binsum, kind="stable")
    place_core = np.empty(NBLK, np.int64)
    place_idx = np.empty(NBLK, np.int64)
    for g in range(BPC):
        grp = order_b[g * CORES:(g + 1) * CORES]
        place_core[grp] = np.arange(len(grp))
        place_idx[grp] = g
    # global slot row of node n (row in kv_full / uv order)
    slotrow = (place_core[bin_of] * NPC + place_idx[bin_of] * 128
               + slot_in)                                    # [NPAD]

    core_of = place_core[bin_of[s_dst]]
    blk_of = place_idx[bin_of[s_dst]]
    gblk = core_of * BPC + blk_of

    cnt = np.zeros((CORES, BPC), dtype=np.int64)
    np.add.at(cnt, (core_of, blk_of), 1)
    T_b = np.maximum(1, np.ceil(cnt.max(axis=0) / 128).astype(np.int64))
    tiles_total = int(T_b.sum())

    blk_starts = np.concatenate([[0], np.cumsum(T_b)])[:-1] * 128
    # re-pairing makes gblk non-contiguous in dst-sorted order: cumcount
    order2 = np.argsort(gblk, kind="stable")
    ks = gblk[order2]
    grp_start = np.concatenate(
        [[0], np.cumsum(np.bincount(ks, minlength=CORES * BPC))])
    pos_in_grp = np.empty(E, dtype=np.int64)
    pos_in_grp[order2] = np.arange(E) - grp_start[ks]

    cap = tiles_total * 128
    ekv = np.zeros((CORES, cap), dtype=np.int32)     # global src node id
    eslot = np.full((CORES, cap), -1.0, dtype=np.float32)

    flat_pos = blk_starts[blk_of] + pos_in_grp
    ekv[core_of, flat_pos] = slotrow[s_src].astype(np.int32)
    eslot[core_of, flat_pos] = slot_in[s_dst].astype(np.float32)

    # transposed one-hot: S2[j, t*128+e] = (eslot[t*128+e] == j), bf16
    s2 = (eslot.reshape(CORES, 1, cap)
          == np.arange(128, dtype=np.float32)[None, :, None])
    s2 = s2.astype(ml_dtypes.bfloat16)
    # [128, tiles_total] partition-major: entry [p, t] = edge t*128+p
    ekv = ekv.reshape(CORES, tiles_total, 128).transpose(0, 2, 1).copy()
    eslot = np.ascontiguousarray(
        eslot.reshape(CORES, tiles_total, 128).transpose(0, 2, 1)
    ).astype(ml_dtypes.bfloat16)

    meta = dict(N=N, C=C, E=E, NPC=NPC, BPC=BPC, NPAD=NPAD,
                T_b=tuple(int(t) for t in T_b), tiles_total=tiles_total)
    arrays = dict(ekv=ekv, eslot=eslot, s2=s2, slotrow=slotrow)
    return meta, arrays


def _prep_weights(inputs):
    """Fold relation transforms, attention scale and skip gates into weights.

    Stored features are pre-scaled: x_stored = (1-a1)*x, h1_stored = (1-a2)*h1,
    so the skip connection becomes a plain add and the projection weights are
    divided by the input scale.
    """
    C = inputs["W1k"].shape[0]
    H, D = inputs["a1"].shape[0], inputs["a1"].shape[1]
    a_s = {l: float(1.0 / (1.0 + np.exp(-float(np.asarray(inputs[f"skip{l}"])))))
           for l in (1, 2)}
    out = {"asig1": a_s[1], "asig2": a_s[2]}
    names = []
    for l in (1, 2):
        a_rel = np.asarray(inputs[f"a{l}"], np.float64)
        m_rel = np.asarray(inputs[f"m{l}"], np.float64)
        p_rel = np.asarray(inputs[f"p{l}"], np.float64)
        A = np.zeros((C, C)); M = np.zeros((C, C))
        for h in range(H):
            A[h * D:(h + 1) * D, h * D:(h + 1) * D] = a_rel[h]
            M[h * D:(h + 1) * D, h * D:(h + 1) * D] = m_rel[h]
        qscale = np.repeat(p_rel / np.sqrt(D), D)
        in_scale = 1.0 - a_s[l]
        Wq = np.asarray(inputs[f"W{l}q"], np.float64) * qscale / in_scale
        Wk = np.asarray(inputs[f"W{l}k"], np.float64) @ A / in_scale
        Wv = np.asarray(inputs[f"W{l}v"], np.float64) @ M / in_scale
        bq = np.asarray(inputs[f"b{l}q"], np.float64) * qscale
        bk = np.asarray(inputs[f"b{l}k"], np.float64) @ A
        bv = np.asarray(inputs[f"b{l}v"], np.float64) @ M
        out_scale = a_s[1] * (1.0 - a_s[2]) if l == 1 else a_s[2]
        Wo = np.asarray(inputs[f"Wo{l}"], np.float64) * out_scale
        boa = np.asarray(inputs[f"bo{l}"], np.float64) * out_scale
        out[f"Wq{l}"] = Wq.astype(np.float16)
        out[f"Wk{l}"] = Wk.astype(np.float16)
        out[f"Wv{l}"] = Wv.astype(np.float16)
        out[f"Wo{l}"] = Wo.astype(np.float16)
        out[f"bq{l}"] = np.broadcast_to(bq.astype(np.float32), (128, C)).copy()
        out[f"bk{l}"] = np.broadcast_to(bk.astype(np.float32), (128, C)).copy()
        out[f"bv{l}"] = np.broadcast_to(bv.astype(np.float32), (128, C)).copy()
        out[f"boa{l}"] = boa.astype(np.float32).reshape(C, 1).copy()
        names += [f"Wq{l}", f"Wk{l}", f"Wv{l}", f"Wo{l}",
                  f"bq{l}", f"bk{l}", f"bv{l}", f"boa{l}"]
    Wlp = np.asarray(inputs["Wlp"], np.float64)
    out["w12"] = np.stack([Wlp[:C, 0], Wlp[C:, 0]], axis=1).astype(np.float16)
    names.append("w12")
    out["names"] = names
    out["blp"] = float(np.asarray(inputs["blp"]).reshape(-1)[0])
    out["xscale"] = 1.0 - a_s[1]
    return out


# ------------------------------------------------------------------- program

def _build_program(meta, asig1, asig2):
    NPC, BPC, NPAD = meta["NPC"], meta["BPC"], meta["NPAD"]
    T_b, tiles_total = meta["T_b"], meta["tiles_total"]
    Tmax = max(T_b)
    col = np.concatenate([[0], np.cumsum(T_b)]).astype(int)
    C = meta["C"]
    kap = (1.0 - asig2, 1.0)  # skip-add scale on stored input, per layer

    nc = bacc.Bacc("TRN2", target_bir_lowering=False, debug=False,
                   num_devices=CORES)

    # --- I/O -------------------------------------------------------------
    xT_in = nc.dram_tensor("xT", [C, NPC], F16, kind="ExternalInput").ap()
    ekv_in = nc.dram_tensor("ekv", [128, tiles_total], I32,
                            kind="ExternalInput").ap()
    eslot_in = nc.dram_tensor("eslot", [128, tiles_total], BF16,
                              kind="ExternalInput").ap()
    s2_in = nc.dram_tensor("s2", [128, tiles_total * 128], BF16,
                           kind="ExternalInput").ap()
    wspec = {}
    for l in (1, 2):
        for n in ("Wq", "Wk", "Wv", "Wo"):
            wspec[f"{n}{l}"] = ([128, C], F16)
        for n in ("bq", "bk", "bv"):
            wspec[f"{n}{l}"] = ([128, C], F32)
        wspec[f"boa{l}"] = ([128, 1], F32)
    wspec["w12"] = ([128, 2], F16)
    w_in = {n: nc.dram_tensor(n, s, d, kind="ExternalInput").ap()
            for n, (s, d) in wspec.items()}
    uv_out = nc.dram_tensor("uv_out", [2, NPC], F32, kind="ExternalOutput").ap()

    # --- DRAM scratch ----------------------------------------------------
    kv_shard = [nc.dram_tensor(f"kv_shard{l}", [NPC, 2 * C], F16,
                               kind="Internal").ap() for l in (0, 1)]
    kv_full = [nc.dram_tensor(f"kv_full{l}", [NPAD, 2 * C], F16,
                              kind="Internal").ap() for l in (0, 1)]

    with tile.TileContext(nc) as tc, ExitStack() as ctx:
        cpool = ctx.enter_context(tc.tile_pool(name="const", bufs=1))
        sb = ctx.enter_context(tc.tile_pool(name="sb", bufs=2))
        sbg = ctx.enter_context(tc.tile_pool(name="sbg", bufs=4))
        psum = ctx.enter_context(tc.tile_pool(name="ps", bufs=2, space="PSUM"))

        # --- constants into SBUF ----------------------------------------
        W = {}
        for n, (s, d) in wspec.items():
            W[n] = cpool.tile(s, d, tag=f"w_{n}", name=f"wt_{n}")
            nc.sync.dma_start(W[n][:], w_in[n][:])
        ekv_sb = cpool.tile([128, tiles_total], I32, tag="ekv")
        nc.sync.dma_start(ekv_sb[:], ekv_in[:])
        eslot_sb = cpool.tile([128, tiles_total], BF16, tag="eslot")
        nc.sync.dma_start(eslot_sb[:], eslot_in[:])

        ident = cpool.tile([128, 128], F32, tag="ident")
        make_identity(nc, ident[:])
        iota_i = cpool.tile([128, Tmax * 128], I32, tag="iota_i")
        nc.gpsimd.iota(iota_i[:], pattern=[[0, Tmax], [1, 128]], base=0,
                       channel_multiplier=0)
        iota_bf = cpool.tile([128, Tmax * 128], BF16, tag="iota_bf")
        nc.vector.tensor_copy(iota_bf[:], iota_i[:])

        xT_all = cpool.tile([128, NPC], F16, tag="xT_all")
        nc.sync.dma_start(xT_all[:], xT_in[:])
        h1T_all = cpool.tile([128, NPC], F16, tag="h1T")
        aggn_all = [cpool.tile([128, NPC], BF16, tag=f"aggn{l}", name=f"aggn{l}")
                    for l in (0, 1)]
        uv_all = cpool.tile([2, NPC], F32, tag="uv")
        q_all = [cpool.tile([128, NPC], F16, tag=f"q_all{l}", name=f"q_all{l}")
                 for l in (0, 1)]

        srcT = [xT_all, h1T_all]

        def proj_block(li, b):
            l = li + 1
            kvs_d, qa = kv_shard[li], q_all[li]
            sl = slice(b * 128, (b + 1) * 128)
            lhs = srcT[li][:, sl]
            if True:
                q_ps = psum.tile([128, C], F32, tag="mm128")
                nc.tensor.matmul(out=q_ps[:], lhsT=lhs, rhs=W[f"Wq{l}"][:],
                                 start=True, stop=True)
                nc.vector.tensor_tensor(out=qa[:, sl], in0=q_ps[:],
                                        in1=W[f"bq{l}"][:], op=OP.add)
                kvs = sb.tile([128, 2 * C], F16, tag="kvs")
                k_ps = psum.tile([128, C], F32, tag="mm128")
                nc.tensor.matmul(out=k_ps[:], lhsT=lhs, rhs=W[f"Wk{l}"][:],
                                 start=True, stop=True)
                nc.vector.tensor_tensor(out=kvs[:, 0:C], in0=k_ps[:],
                                        in1=W[f"bk{l}"][:], op=OP.add)
                v_ps = psum.tile([128, C], F32, tag="mm128")
                nc.tensor.matmul(out=v_ps[:], lhsT=lhs, rhs=W[f"Wv{l}"][:],
                                 start=True, stop=True)
                nc.vector.tensor_tensor(out=kvs[:, C:2 * C], in0=v_ps[:],
                                        in1=W[f"bv{l}"][:], op=OP.add)
                nc.sync.dma_start(kvs_d[sl, :], kvs[:])

        def allgather(li):
            nc.gpsimd.collective_compute(
                "AllGather", OP.bypass,
                replica_groups=[list(range(CORES))],
                ins=[kv_shard[li][:]], outs=[kv_full[li][:]])

        def edge_block(li, b):
            l = li + 1
            kvf, qa = kv_full[li], q_all[li]
            if True:
                T = T_b[b]
                c0 = int(col[b])
                kvg = sbg.tile([128, Tmax * 2 * C], F16, tag="kvg")
                for t in range(T):
                    nc.gpsimd.indirect_dma_start(
                        out=kvg[:, t * 256:(t + 1) * 256], out_offset=None,
                        in_=kvf,
                        in_offset=bass.IndirectOffsetOnAxis(
                            ap=ekv_sb[:, c0 + t:c0 + t + 1], axis=0))
                S2 = sb.tile([128, Tmax * 128], BF16, tag="S2")
                nc.sync.dma_start(
                    S2[:, :T * 128],
                    s2_in[:, c0 * 128:(c0 + T) * 128])
                qg = sb.tile([128, Tmax * C], F16, tag="qg")
                for t in range(T):
                    qg_ps = psum.tile([128, C], F32, tag="qg")
                    nc.tensor.matmul(out=qg_ps[:],
                                     lhsT=S2[:, t * 128:(t + 1) * 128],
                                     rhs=qa[:, b * 128:(b + 1) * 128],
                                     start=True, stop=True)
                    nc.vector.tensor_copy(qg[:, t * 128:(t + 1) * 128],
                                          qg_ps[:])
                S = sb.tile([128, Tmax * 128], BF16, tag="S")
                nc.vector.tensor_tensor(
                    out=_apn(S[:], [[128, T], [1, 128]]),
                    in0=_apn(iota_bf[:], [[128, T], [1, 128]]),
                    in1=_expand_last(eslot_sb[:, c0:c0 + T], 128),
                    op=OP.is_equal)
                prod = sb.tile([128, Tmax * C], F16, tag="prod")
                nc.vector.tensor_tensor(
                    out=_apn(prod[:], [[128, T], [1, 128]]),
                    in0=_apn(kvg[:], [[256, T], [1, 128]]),
                    in1=_apn(qg[:], [[128, T], [1, 128]]), op=OP.mult)
                alpha = sb.tile([128, Tmax * 4], F32, tag="alpha")
                nc.vector.tensor_reduce(
                    out=alpha[:, :T * 4],
                    in_=_apn(prod[:], [[32, T * 4], [1, 32]]),
                    axis=mybir.AxisListType.X, op=OP.add)
                ex = sb.tile([128, Tmax * 4], BF16, tag="ex")
                nc.scalar.activation(ex[:, :T * 4], alpha[:, :T * 4], AF.Exp)
                evex = sb.tile([128, Tmax * 132], BF16, tag="evex")
                nc.scalar.activation(_apn(evex[:], [[132, T], [1, 4]]),
                                     alpha[:, :T * 4], AF.Exp)
                nc.vector.tensor_tensor(
                    out=_apn(evex[:], [[132, T], [32, 4], [1, 32]], off=4),
                    in0=_apn(kvg[:], [[256, T], [32, 4], [1, 32]], off=128),
                    in1=_apn(ex[:], [[4, T], [1, 4], [0, 32]]), op=OP.mult)
                agg_ps = psum.tile([128, 132], F32, tag="mm132")
                for t in range(T):
                    nc.tensor.matmul(out=agg_ps[:],
                                     lhsT=S[:, t * 128:(t + 1) * 128],
                                     rhs=evex[:, t * 132:(t + 1) * 132],
                                     start=(t == 0), stop=(t == T - 1))
                den = sb.tile([128, 4], F32, tag="den")
                nc.vector.tensor_scalar_add(den[:], agg_ps[:, 0:4], EPS)
                rd = sb.tile([128, 4], F32, tag="rd")
                nc.vector.reciprocal(rd[:], den[:])
                nc.vector.tensor_tensor(
                    out=_apn(aggn_all[li][:], [[32, 4], [1, 32]], off=b * 128),
                    in0=_apn(agg_ps[:], [[32, 4], [1, 32]], off=4),
                    in1=_expand_last(rd[:], 32), op=OP.mult)

        def pass2_block(li, b):
            l = li + 1
            if True:
                sl = slice(b * 128, (b + 1) * 128)
                g = sb.tile([128, C], F32, tag="g")
                nc.scalar.activation(g[:], aggn_all[li][:, sl], AF.Gelu)
                gT_ps = psum.tile([128, C], F32, tag="tr")
                nc.tensor.transpose(out=gT_ps[:], in_=g[:], identity=ident[:])
                gT = sb.tile([128, C], BF16, tag="gTs")
                nc.vector.tensor_copy(gT[:], gT_ps[:])
                hm_ps = psum.tile([128, C], F32, tag="mm128")
                nc.tensor.matmul(out=hm_ps[:], lhsT=W[f"Wo{l}"][:], rhs=gT[:],
                                 start=True, stop=True)
                if l == 1:
                    zt_ap = h1T_all[:, sl]
                else:
                    zt = sb.tile([128, C], F16, tag="zt")
                    zt_ap = zt[:]
                nc.vector.scalar_tensor_tensor(
                    out=zt_ap, in0=srcT[li][:, sl], scalar=kap[li],
                    in1=hm_ps[:], op0=OP.mult, op1=OP.add)
                nc.vector.tensor_tensor(
                    out=zt_ap, in0=zt_ap,
                    in1=_apn(W[f"boa{l}"][:], [[0, 128]]), op=OP.add)
                if l == 2:
                    uv_ps = psum.tile([128, C], F32, tag="mm128")
                    nc.tensor.matmul(out=uv_ps[0:2, :], lhsT=W["w12"][:],
                                     rhs=zt_ap, start=True, stop=True)
                    nc.vector.tensor_copy(uv_all[:, sl], uv_ps[0:2, :])

        # interleaved schedule: layer-1 pass-2 and layer-2 projections ride
        # inside the layer-1 edge loop so AllGather 2 fires with minimal tail.
        for b in range(BPC):
            proj_block(0, b)
        allgather(0)
        for b in range(BPC):
            edge_block(0, b)
            pass2_block(0, b)
            proj_block(1, b)
        allgather(1)
        for b in range(BPC):
            edge_block(1, b)
            pass2_block(1, b)
        nc.sync.dma_start(uv_out, uv_all[:])

    nc.compile()
    return nc


_CACHE = {}


def _get_program(meta, asig1, asig2, blp):
    key = (meta["N"], meta["E"], meta["T_b"], asig1, asig2)
    if key not in _CACHE:
        _CACHE[key] = _build_program(meta, asig1, asig2)
    return _CACHE[key]


def make_in_maps(inputs):
    inputs = {k: np.asarray(v) for k, v in inputs.items()}
    meta, arrays = _host_prep(np.asarray(inputs["x"], np.float32),
                              inputs["edge_index"])
    w = _prep_weights(inputs)
    N, C, NPC = meta["N"], meta["C"], meta["NPC"]
    xpad = np.zeros((meta["NPAD"], C), dtype=np.float64)
    xpad[:N] = np.asarray(inputs["x"], np.float64)
    # column order: global slot s holds node gorder[s]
    gorder = np.empty(meta["NPAD"], np.int64)
    gorder[arrays["slotrow"]] = np.arange(meta["NPAD"])
    xT_full = np.ascontiguousarray(
        (xpad * w["xscale"]).T[:, gorder]).astype(np.float16)
    in_maps = []
    for c in range(CORES):
        m = dict(xT=np.ascontiguousarray(xT_full[:, c * NPC:(c + 1) * NPC]),
                 ekv=arrays["ekv"][c], eslot=arrays["eslot"][c],
                 s2=arrays["s2"][c])
        for n in w["names"]:
            m[n] = w[n]
        in_maps.append(m)
    meta["slotrow"] = arrays["slotrow"]
    return meta, w, in_maps


def assemble(meta, results, inputs, blp):
    u = np.concatenate([results[c]["uv_out"] for c in range(CORES)], axis=1)
    sr = meta["slotrow"]
    u1, u2 = u[0][sr], u[1][sr]          # back to node-id order
    pe, ne = inputs["pos_edge_index"], inputs["neg_edge_index"]
    pos = u1[pe[0]] + u2[pe[1]] + np.float32(blp)
    neg = u1[ne[0]] + u2[ne[1]] + np.float32(blp)
    return pos.astype(np.float32), neg.astype(np.float32)


def kernel(**inputs):
    meta, w, in_maps = make_in_maps(inputs)
    nc = _get_program(meta, w["asig1"], w["asig2"], w["blp"])
    res = bass_utils.run_bass_kernel_spmd(nc, in_maps,
                                          core_ids=list(range(CORES)))
    return assemble(meta, res.results, inputs, w["blp"])


# revision 22
# speedup vs baseline: 2.8846x; 1.0059x over previous
"""HGT link predictor on 8 Trainium2 NeuronCores (Bass/Tile SPMD kernel).

Strategy (hardcoded for nn_HGTLinkPredictor, N=50000 E=800000 P=100000 C=128 H=4 D=32):
 - Shard dst nodes (and their incoming edges) across 8 cores in contiguous
   128-node blocks. Edges sorted by dst on host.
 - Per layer: each core computes q/k/v projections for its node shard from a
   host-transposed feature tile (relation transforms, attention scale and the
   sigmoid-skip coefficients are all folded into the weights on host), writes
   packed [k|v] fp16 rows, AllGathers them so every core can fetch k/v of any
   src node with a single 512B-descriptor indirect DMA per edge (one packed
   row instead of separate k/v/q fetches).
 - q[dst] rows never leave SBUF: per 128-edge tile a one-hot matmul
   (S2^T @ Q_block) on the tensor engine expands the block's q rows to edge
   slots, with the transposed one-hot streamed from a host-precomputed table.
 - Attention logits via fp16 elementwise mult + segmented reduce; exp on the
   scalar engine into a fused [ex | v*ex] bf16 tile so a single 132-column
   matmul per 128-edge tile accumulates both the softmax denominator and the
   weighted value sum in PSUM.
 - Gelu + output projection run as a deferred second pass (one activation
   table load), producing transposed features directly so layer-2 projections
   need no PE transposes. Link decode is one tiny matmul per block; final
   per-edge logits are assembled host-side from per-node partial sums.
"""

import math
import numpy as np
import ml_dtypes
from contextlib import ExitStack

import concourse.bass as bass
import concourse.tile as tile
from concourse import bacc, mybir
from concourse import bass_utils
from concourse.masks import make_identity

F32 = mybir.dt.float32
F16 = mybir.dt.float16
BF16 = mybir.dt.bfloat16
I32 = mybir.dt.int32
AF = mybir.ActivationFunctionType
OP = mybir.AluOpType

CORES = 8
EPS = 1e-30
def _expand_last(ap, n):
    """Append a step-0 (broadcast) innermost dim of size n to an AP."""
    new = [list(p) for p in ap.ap] + [[0, n]]
    return bass.AP(ap.tensor, ap.offset, new)


def _apn(ap, dims, off=0):
    """AP keeping the partition dim of `ap` but custom free-dim pattern."""
    return bass.AP(ap.tensor, ap.offset + off,
                   [list(ap.ap[0])] + [list(d) for d in dims])


# ----------------------------------------------------------------- host prep

def _host_prep(x, edge_index):
    N, C = x.shape
    E = edge_index.shape[1]

    NPC = int(math.ceil(N / (CORES * 128))) * 128   # nodes per core (padded)
    BPC = NPC // 128                                # blocks per core
    NPAD = NPC * CORES

    src = edge_index[0].astype(np.int64)
    dst = edge_index[1].astype(np.int64)
    order = np.argsort(dst, kind="stable")
    s_src, s_dst = src[order], dst[order]

    # Balanced dst partition: assign nodes to blocks (exactly 128 each) by
    # greedy degree balancing, then pair blocks of similar load into
    # (core, index) groups — T_b is the max over cores at each index.
    import heapq
    NBLK = NPAD // 128
    deg = np.bincount(dst, minlength=NPAD)
    order_n = np.argsort(-deg, kind="stable")
    heap = [(0, 0, b) for b in range(NBLK)]
    heapq.heapify(heap)
    bin_of = np.empty(NPAD, np.int64)
    slot_in = np.empty(NPAD, np.int64)
    for n in order_n:
        s, cb, b = heapq.heappop(heap)
        bin_of[n] = b
        slot_in[n] = cb
        if cb + 1 < 128:
            heapq.heappush(heap, (s + int(deg[n]), cb + 1, b))
    binsum = np.bincount(bin_of, weights=deg.astype(np.float64),
                         minlength=NBLK)
    order_b = np.argsort(-binsum, kind="stable")
    place_core = np.empty(NBLK, np.int64)
    place_idx = np.empty(NBLK, np.int64)
    for g in range(BPC):
        grp = order_b[g * CORES:(g + 1) * CORES]
        place_core[grp] = np.arange(len(grp))
        place_idx[grp] = g
    # global slot row of node n (row in kv_full / uv order)
    slotrow = (place_core[bin_of] * NPC + place_idx[bin_of] * 128
               + slot_in)                                    # [NPAD]

    core_of = place_core[bin_of[s_dst]]
    blk_of = place_idx[bin_of[s_dst]]
    gblk = core_of * BPC + blk_of

    cnt = np.zeros((CORES, BPC), dtype=np.int64)
    np.add.at(cnt, (core_of, blk_of), 1)
    T_b = np.maximum(1, np.ceil(cnt.max(axis=0) / 128).astype(np.int64))
    tiles_total = int(T_b.sum())

    blk_starts = np.concatenate([[0], np.cumsum(T_b)])[:-1] * 128
    # re-pairing makes gblk non-contiguous in dst-sorted order: cumcount
    order2 = np.argsort(gblk, kind="stable")
    ks = gblk[order2]
    grp_start = np.concatenate(
        [[0], np.cumsum(np.bincount(ks, minlength=CORES * BPC))])
    pos_in_grp = np.empty(E, dtype=np.int64)
    pos_in_grp[order2] = np.arange(E) - grp_start[ks]

    cap = tiles_total * 128
    ekv = np.zeros((CORES, cap), dtype=np.int32)     # global src node id
    eslot = np.full((CORES, cap), -1.0, dtype=np.float32)

    flat_pos = blk_starts[blk_of] + pos_in_grp
    ekv[core_of, flat_pos] = slotrow[s_src].astype(np.int32)
    eslot[core_of, flat_pos] = slot_in[s_dst].astype(np.float32)

    # transposed one-hot: S2[j, t*128+e] = (eslot[t*128+e] == j), bf16
    s2 = (eslot.reshape(CORES, 1, cap)
          == np.arange(128, dtype=np.float32)[None, :, None])
    s2 = s2.astype(ml_dtypes.bfloat16)
    # [128, tiles_total] partition-major: entry [p, t] = edge t*128+p
    ekv = ekv.reshape(CORES, tiles_total, 128).transpose(0, 2, 1).copy()
    eslot = np.ascontiguousarray(
        eslot.reshape(CORES, tiles_total, 128).transpose(0, 2, 1)
    ).astype(ml_dtypes.bfloat16)

    meta = dict(N=N, C=C, E=E, NPC=NPC, BPC=BPC, NPAD=NPAD,
                T_b=tuple(int(t) for t in T_b), tiles_total=tiles_total)
    arrays = dict(ekv=ekv, eslot=eslot, s2=s2, slotrow=slotrow)
    return meta, arrays


def _prep_weights(inputs):
    """Fold relation transforms, attention scale and skip gates into weights.

    Stored features are pre-scaled: x_stored = (1-a1)*x, h1_stored = (1-a2)*h1,
    so the skip connection becomes a plain add and the projection weights are
    divided by the input scale.
    """
    C = inputs["W1k"].shape[0]
    H, D = inputs["a1"].shape[0], inputs["a1"].shape[1]
    a_s = {l: float(1.0 / (1.0 + np.exp(-float(np.asarray(inputs[f"skip{l}"])))))
           for l in (1, 2)}
    out = {"asig1": a_s[1], "asig2": a_s[2]}
    names = []
    for l in (1, 2):
        a_rel = np.asarray(inputs[f"a{l}"], np.float64)
        m_rel = np.asarray(inputs[f"m{l}"], np.float64)
        p_rel = np.asarray(inputs[f"p{l}"], np.float64)
        A = np.zeros((C, C)); M = np.zeros((C, C))
        for h in range(H):
            A[h * D:(h + 1) * D, h * D:(h + 1) * D] = a_rel[h]
            M[h * D:(h + 1) * D, h * D:(h + 1) * D] = m_rel[h]
        qscale = np.repeat(p_rel / np.sqrt(D), D)
        in_scale = 1.0 - a_s[l]
        Wq = np.asarray(inputs[f"W{l}q"], np.float64) * qscale / in_scale
        Wk = np.asarray(inputs[f"W{l}k"], np.float64) @ A / in_scale
        Wv = np.asarray(inputs[f"W{l}v"], np.float64) @ M / in_scale
        bq = np.asarray(inputs[f"b{l}q"], np.float64) * qscale
        bk = np.asarray(inputs[f"b{l}k"], np.float64) @ A
        bv = np.asarray(inputs[f"b{l}v"], np.float64) @ M
        out_scale = a_s[1] * (1.0 - a_s[2]) if l == 1 else a_s[2]
        Wo = np.asarray(inputs[f"Wo{l}"], np.float64) * out_scale
        boa = np.asarray(inputs[f"bo{l}"], np.float64) * out_scale
        out[f"Wq{l}"] = Wq.astype(np.float16)
        out[f"Wk{l}"] = Wk.astype(np.float16)
        out[f"Wv{l}"] = Wv.astype(np.float16)
        out[f"Wo{l}"] = Wo.astype(np.float16)
        out[f"bq{l}"] = np.broadcast_to(bq.astype(np.float32), (128, C)).copy()
        out[f"bk{l}"] = np.broadcast_to(bk.astype(np.float32), (128, C)).copy()
        out[f"bv{l}"] = np.broadcast_to(bv.astype(np.float32), (128, C)).copy()
        out[f"boa{l}"] = boa.astype(np.float32).reshape(C, 1).copy()
        names += [f"Wq{l}", f"Wk{l}", f"Wv{l}", f"Wo{l}",
                  f"bq{l}", f"bk{l}", f"bv{l}", f"boa{l}"]
    Wlp = np.asarray(inputs["Wlp"], np.float64)
    out["w12"] = np.stack([Wlp[:C, 0], Wlp[C:, 0]], axis=1).astype(np.float16)
    names.append("w12")
    out["names"] = names
    out["blp"] = float(np.asarray(inputs["blp"]).reshape(-1)[0])
    out["xscale"] = 1.0 - a_s[1]
    return out


# ------------------------------------------------------------------- program

def _build_program(meta, asig1, asig2):
    NPC, BPC, NPAD = meta["NPC"], meta["BPC"], meta["NPAD"]
    T_b, tiles_total = meta["T_b"], meta["tiles_total"]
    Tmax = max(T_b)
    col = np.concatenate([[0], np.cumsum(T_b)]).astype(int)
    C = meta["C"]
    kap = (1.0 - asig2, 1.0)  # skip-add scale on stored input, per layer

    nc = bacc.Bacc("TRN2", target_bir_lowering=False, debug=False,
                   num_devices=CORES)

    # --- I/O -------------------------------------------------------------
    xT_in = nc.dram_tensor("xT", [C, NPC], F16, kind="ExternalInput").ap()
    ekv_in = nc.dram_tensor("ekv", [128, tiles_total], I32,
                            kind="ExternalInput").ap()
    eslot_in = nc.dram_tensor("eslot", [128, tiles_total], BF16,
                              kind="ExternalInput").ap()
    s2_in = nc.dram_tensor("s2", [128, tiles_total * 128], BF16,
                           kind="ExternalInput").ap()
    wspec = {}
    for l in (1, 2):
        for n in ("Wq", "Wk", "Wv", "Wo"):
            wspec[f"{n}{l}"] = ([128, C], F16)
        for n in ("bq", "bk", "bv"):
            wspec[f"{n}{l}"] = ([128, C], F32)
        wspec[f"boa{l}"] = ([128, 1], F32)
    wspec["w12"] = ([128, 2], F16)
    w_in = {n: nc.dram_tensor(n, s, d, kind="ExternalInput").ap()
            for n, (s, d) in wspec.items()}
    uv_out = nc.dram_tensor("uv_out", [2, NPC], F32, kind="ExternalOutput").ap()

    # --- DRAM scratch ----------------------------------------------------
    kv_shard = [nc.dram_tensor(f"kv_shard{l}", [NPC, 2 * C], F16,
                               kind="Internal").ap() for l in (0, 1)]
    kv_full = [nc.dram_tensor(f"kv_full{l}", [NPAD, 2 * C], F16,
                              kind="Internal").ap() for l in (0, 1)]

    with tile.TileContext(nc) as tc, ExitStack() as ctx:
        cpool = ctx.enter_context(tc.tile_pool(name="const", bufs=1))
        sb = ctx.enter_context(tc.tile_pool(name="sb", bufs=2))
        sbg = ctx.enter_context(tc.tile_pool(name="sbg", bufs=5))
        psum = ctx.enter_context(tc.tile_pool(name="ps", bufs=2, space="PSUM"))

        # --- constants into SBUF ----------------------------------------
        W = {}
        for n, (s, d) in wspec.items():
            W[n] = cpool.tile(s, d, tag=f"w_{n}", name=f"wt_{n}")
            nc.sync.dma_start(W[n][:], w_in[n][:])
        ekv_sb = cpool.tile([128, tiles_total], I32, tag="ekv")
        nc.sync.dma_start(ekv_sb[:], ekv_in[:])
        eslot_sb = cpool.tile([128, tiles_total], BF16, tag="eslot")
        nc.sync.dma_start(eslot_sb[:], eslot_in[:])

        ident = cpool.tile([128, 128], F32, tag="ident")
        make_identity(nc, ident[:])
        iota_i = cpool.tile([128, Tmax * 128], I32, tag="iota_i")
        nc.gpsimd.iota(iota_i[:], pattern=[[0, Tmax], [1, 128]], base=0,
                       channel_multiplier=0)
        iota_bf = cpool.tile([128, Tmax * 128], BF16, tag="iota_bf")
        nc.vector.tensor_copy(iota_bf[:], iota_i[:])

        xT_all = cpool.tile([128, NPC], F16, tag="xT_all")
        nc.sync.dma_start(xT_all[:], xT_in[:])
        h1T_all = cpool.tile([128, NPC], F16, tag="h1T")
        aggn_all = [cpool.tile([128, NPC], BF16, tag=f"aggn{l}", name=f"aggn{l}")
                    for l in (0, 1)]
        uv_all = cpool.tile([2, NPC], F32, tag="uv")
        q_all = [cpool.tile([128, NPC], F16, tag=f"q_all{l}", name=f"q_all{l}")
                 for l in (0, 1)]

        srcT = [xT_all, h1T_all]

        def proj_block(li, b):
            l = li + 1
            kvs_d, qa = kv_shard[li], q_all[li]
            sl = slice(b * 128, (b + 1) * 128)
            lhs = srcT[li][:, sl]
            if True:
                q_ps = psum.tile([128, C], F32, tag="mm128")
                nc.tensor.matmul(out=q_ps[:], lhsT=lhs, rhs=W[f"Wq{l}"][:],
                                 start=True, stop=True)
                nc.vector.tensor_tensor(out=qa[:, sl], in0=q_ps[:],
                                        in1=W[f"bq{l}"][:], op=OP.add)
                kvs = sb.tile([128, 2 * C], F16, tag="kvs")
                k_ps = psum.tile([128, C], F32, tag="mm128")
                nc.tensor.matmul(out=k_ps[:], lhsT=lhs, rhs=W[f"Wk{l}"][:],
                                 start=True, stop=True)
                nc.vector.tensor_tensor(out=kvs[:, 0:C], in0=k_ps[:],
                                        in1=W[f"bk{l}"][:], op=OP.add)
                v_ps = psum.tile([128, C], F32, tag="mm128")
                nc.tensor.matmul(out=v_ps[:], lhsT=lhs, rhs=W[f"Wv{l}"][:],
                                 start=True, stop=True)
                nc.vector.tensor_tensor(out=kvs[:, C:2 * C], in0=v_ps[:],
                                        in1=W[f"bv{l}"][:], op=OP.add)
                nc.sync.dma_start(kvs_d[sl, :], kvs[:])

        def allgather(li):
            nc.gpsimd.collective_compute(
                "AllGather", OP.bypass,
                replica_groups=[list(range(CORES))],
                ins=[kv_shard[li][:]], outs=[kv_full[li][:]])

        def edge_block(li, b):
            l = li + 1
            kvf, qa = kv_full[li], q_all[li]
            if True:
                T = T_b[b]
                c0 = int(col[b])
                kvg = sbg.tile([128, Tmax * 2 * C], F16, tag="kvg")
                for t in range(T):
                    nc.gpsimd.indirect_dma_start(
                        out=kvg[:, t * 256:(t + 1) * 256], out_offset=None,
                        in_=kvf,
                        in_offset=bass.IndirectOffsetOnAxis(
                            ap=ekv_sb[:, c0 + t:c0 + t + 1], axis=0))
                S2 = sb.tile([128, Tmax * 128], BF16, tag="S2")
                nc.sync.dma_start(
                    S2[:, :T * 128],
                    s2_in[:, c0 * 128:(c0 + T) * 128])
                qg = sb.tile([128, Tmax * C], F16, tag="qg")
                for t in range(T):
                    qg_ps = psum.tile([128, C], F32, tag="qg")
                    nc.tensor.matmul(out=qg_ps[:],
                                     lhsT=S2[:, t * 128:(t + 1) * 128],
                                     rhs=qa[:, b * 128:(b + 1) * 128],
                                     start=True, stop=True)
                    nc.vector.tensor_copy(qg[:, t * 128:(t + 1) * 128],
                                          qg_ps[:])
                S = sb.tile([128, Tmax * 128], BF16, tag="S")
                nc.vector.tensor_tensor(
                    out=_apn(S[:], [[128, T], [1, 128]]),
                    in0=_apn(iota_bf[:], [[128, T], [1, 128]]),
                    in1=_expand_last(eslot_sb[:, c0:c0 + T], 128),
                    op=OP.is_equal)
                prod = sb.tile([128, Tmax * C], F16, tag="prod")
                nc.vector.tensor_tensor(
                    out=_apn(prod[:], [[128, T], [1, 128]]),
                    in0=_apn(kvg[:], [[256, T], [1, 128]]),
                    in1=_apn(qg[:], [[128, T], [1, 128]]), op=OP.mult)
                alpha = sb.tile([128, Tmax * 4], F32, tag="alpha")
                nc.vector.tensor_reduce(
                    out=alpha[:, :T * 4],
                    in_=_apn(prod[:], [[32, T * 4], [1, 32]]),
                    axis=mybir.AxisListType.X, op=OP.add)
                ex = sb.tile([128, Tmax * 4], BF16, tag="ex")
                nc.scalar.activation(ex[:, :T * 4], alpha[:, :T * 4], AF.Exp)
                evex = sb.tile([128, Tmax * 132], BF16, tag="evex")
                nc.scalar.activation(_apn(evex[:], [[132, T], [1, 4]]),
                                     alpha[:, :T * 4], AF.Exp)
                nc.vector.tensor_tensor(
                    out=_apn(evex[:], [[132, T], [32, 4], [1, 32]], off=4),
                    in0=_apn(kvg[:], [[256, T], [32, 4], [1, 32]], off=128),
                    in1=_apn(ex[:], [[4, T], [1, 4], [0, 32]]), op=OP.mult)
                agg_ps = psum.tile([128, 132], F32, tag="mm132")
                for t in range(T):
                    nc.tensor.matmul(out=agg_ps[:],
                                     lhsT=S[:, t * 128:(t + 1) * 128],
                                     rhs=evex[:, t * 132:(t + 1) * 132],
                                     start=(t == 0), stop=(t == T - 1))
                den = sb.tile([128, 4], F32, tag="den")
                nc.vector.tensor_scalar_add(den[:], agg_ps[:, 0:4], EPS)
                rd = sb.tile([128, 4], F32, tag="rd")
                nc.vector.reciprocal(rd[:], den[:])
                nc.vector.tensor_tensor(
                    out=_apn(aggn_all[li][:], [[32, 4], [1, 32]], off=b * 128),
                    in0=_apn(agg_ps[:], [[32, 4], [1, 32]], off=4),
                    in1=_expand_last(rd[:], 32), op=OP.mult)

        def pass2_block(li, b):
            l = li + 1
            if True:
                sl = slice(b * 128, (b + 1) * 128)
                g = sb.tile([128, C], F32, tag="g")
                nc.scalar.activation(g[:], aggn_all[li][:, sl], AF.Gelu)
                gT_ps = psum.tile([128, C], F32, tag="tr")
                nc.tensor.transpose(out=gT_ps[:], in_=g[:], identity=ident[:])
                gT = sb.tile([128, C], BF16, tag="gTs")
                nc.vector.tensor_copy(gT[:], gT_ps[:])
                hm_ps = psum.tile([128, C], F32, tag="mm128")
                nc.tensor.matmul(out=hm_ps[:], lhsT=W[f"Wo{l}"][:], rhs=gT[:],
                                 start=True, stop=True)
                if l == 1:
                    zt_ap = h1T_all[:, sl]
                else:
                    zt = sb.tile([128, C], F16, tag="zt")
                    zt_ap = zt[:]
                nc.vector.scalar_tensor_tensor(
                    out=zt_ap, in0=srcT[li][:, sl], scalar=kap[li],
                    in1=hm_ps[:], op0=OP.mult, op1=OP.add)
                nc.vector.tensor_tensor(
                    out=zt_ap, in0=zt_ap,
                    in1=_apn(W[f"boa{l}"][:], [[0, 128]]), op=OP.add)
                if l == 2:
                    uv_ps = psum.tile([128, C], F32, tag="mm128")
                    nc.tensor.matmul(out=uv_ps[0:2, :], lhsT=W["w12"][:],
                                     rhs=zt_ap, start=True, stop=True)
                    nc.vector.tensor_copy(uv_all[:, sl], uv_ps[0:2, :])

        # interleaved schedule: layer-1 pass-2 and layer-2 projections ride
        # inside the layer-1 edge loop so AllGather 2 fires with minimal tail.
        for b in range(BPC):
            proj_block(0, b)
        allgather(0)
        for b in range(BPC):
            edge_block(0, b)
            pass2_block(0, b)
            proj_block(1, b)
        allgather(1)
        for b in range(BPC):
            edge_block(1, b)
            pass2_block(1, b)
        nc.sync.dma_start(uv_out, uv_all[:])

    nc.compile()
    return nc


_CACHE = {}


def _get_program(meta, asig1, asig2, blp):
    key = (meta["N"], meta["E"], meta["T_b"], asig1, asig2)
    if key not in _CACHE:
        _CACHE[key] = _build_program(meta, asig1, asig2)
    return _CACHE[key]


def make_in_maps(inputs):
    inputs = {k: np.asarray(v) for k, v in inputs.items()}
    meta, arrays = _host_prep(np.asarray(inputs["x"], np.float32),
                              inputs["edge_index"])
    w = _prep_weights(inputs)
    N, C, NPC = meta["N"], meta["C"], meta["NPC"]
    xpad = np.zeros((meta["NPAD"], C), dtype=np.float64)
    xpad[:N] = np.asarray(inputs["x"], np.float64)
    # column order: global slot s holds node gorder[s]
    gorder = np.empty(meta["NPAD"], np.int64)
    gorder[arrays["slotrow"]] = np.arange(meta["NPAD"])
    xT_full = np.ascontiguousarray(
        (xpad * w["xscale"]).T[:, gorder]).astype(np.float16)
    in_maps = []
    for c in range(CORES):
        m = dict(xT=np.ascontiguousarray(xT_full[:, c * NPC:(c + 1) * NPC]),
                 ekv=arrays["ekv"][c], eslot=arrays["eslot"][c],
                 s2=arrays["s2"][c])
        for n in w["names"]:
            m[n] = w[n]
        in_maps.append(m)
    meta["slotrow"] = arrays["slotrow"]
    return meta, w, in_maps


def assemble(meta, results, inputs, blp):
    u = np.concatenate([results[c]["uv_out"] for c in range(CORES)], axis=1)
    sr = meta["slotrow"]
    u1, u2 = u[0][sr], u[1][sr]          # back to node-id order
    pe, ne = inputs["pos_edge_index"], inputs["neg_edge_index"]
    pos = u1[pe[0]] + u2[pe[1]] + np.float32(blp)
    neg = u1[ne[0]] + u2[ne[1]] + np.float32(blp)
    return pos.astype(np.float32), neg.astype(np.float32)


def kernel(**inputs):
    meta, w, in_maps = make_in_maps(inputs)
    nc = _get_program(meta, w["asig1"], w["asig2"], w["blp"])
    res = bass_utils.run_bass_kernel_spmd(nc, in_maps,
                                          core_ids=list(range(CORES)))
    return assemble(meta, res.results, inputs, w["blp"])
